# revision 1
# baseline (speedup 1.0000x reference)
"""AttentionCritic Trainium2 kernel — 8-core SPMD, no collectives.

Math restructuring (exact up to fp assoc.):
  mask[i,j] = (|x_i-x_j|<=4)&(|y_i-y_j|<=2)&(j>i)
  C = [obs, action];  q/k/v = (C@W{q,k,v}+b)@Wi{q,k,v}+bi
  S_h = q_h k_h^T / 12  (shared over agents); E_h = exp(S_h)  (softmax ratio
  is shift-invariant; |S| small enough that exp is safely fp32)
  D[i,h,j] = sum_k E_h[j,k] mask[i,k];  R = mask/max(D,1e-9)
  W[i,h,k] = mask[i,k] * sum_j R[i,h,j] E_h[j,k]
  ctxs[i,h] = sum_k W[i,h,k] v_h[k]    (= masked sum_j of attention rows)
  h_i = ctxs[i] @ Wo_proj @ W_O + n_i * (bo_proj @ W_O);  Q = V + A - mean(A)

Phase A (through E/E^T/v) replicated on all 8 cores; phase B data-parallel
over agents via a per-core one-hot selector selT (only per-core input).

DMA strategy: host packs weights into contiguous "blob" arrays laid out
exactly as the SBUF tiles (partition-major), one DMA per blob, ordered by
first consumer; a shared HWDGE generator costs ~625ns per DMA instruction
so instruction count matters more than anything; small/broadcast loads go
through the independent SWDGE (gpsimd) path.
"""

import sys

for _p in ("/opt/trn_rl_repo",):
    if _p not in sys.path:
        sys.path.append(_p)

import contextlib

import numpy as np

import concourse.bass as bass
import concourse.bacc as bacc
import concourse.mybir as mybir
from concourse.tile import TileContext
from concourse import bass_utils

N, HID, ACT, NH = 256, 128, 5, 4
D, E, HD = 144, 576, 144
NCORES = 8
SH = N // NCORES  # 32
F32 = mybir.dt.float32
F32R = mybir.dt.float32r
I32 = mybir.dt.int32
SCALE = 1.0 / 12.0
EC = [(0, 128), (128, 128), (256, 128), (384, 128), (512, 64)]
DC = [(0, 128), (128, 16)]
HC = []
for _h in range(NH):
    HC += [(HD * _h, 128), (HD * _h + 128, 16)]

# ---- blob column layouts (host packing must match kernel slicing) ----
# blobA [128]: state0(2) state1(2) hiddenT(256) actionT(256) wenc(16) id128(128)
A_ST0, A_ST1, A_HT, A_AT, A_WENC = 0, 2, 4, 4 + 256, 4 + 512
A_ID = A_WENC + 16
A_COLS = A_ID + 128
# blobB [128]: wqkvB(3*576) baB(4*3) bbB(4*2)
B_WQKV, B_BA, B_BB = 0, 3 * E, 3 * E + 12
B_COLS = B_BB + 8
# blobC/D [128]: padded Wi{q,k} rows 0:512 as 4 blocks of 640
#   col order: 4x[head-main 128] + tailsA[h0:16,pad16,h1:16,pad16] + tailsB[h2,h3]
EP = 640
W4P_COLS = 4 * EP
# blobE [128]: Wiv rows 0:512 as 4 blocks of 576
W4_COLS = 4 * E
QKM = [(0, 128), (128, 128), (256, 128), (384, 128), (512, 64), (576, 64)]
# blobF [128]: wo8big(4*576) wOB(4*144) sel2(2*32) wva128(6)
F_WO8, F_WO, F_SEL, F_WVA = 0, 4 * E, 4 * E + 4 * D, 4 * E + 4 * D + 64
F_COLS = F_WVA + 6
# blobS1 [16]: wqkvS(3*576) bbS(4*2) wvaS(6) benc(1)   (early)
S_WQKV, S_BB, S_WVA, S_BENC = 0, 3 * E, 3 * E + 8, 3 * E + 14
S_COLS = S_BENC + 1
# blobS2 [16]: wo8small(4*576)                          (late)
S2_COLS = 4 * E
# blobT [64]: wiqT(640) wikT(640) wivT(576) wOT(144) baT(3) bbTA(2) bbTB(2)
T_WIQ, T_WIK, T_WIV = 0, EP, 2 * EP
T_WO, T_BA = 2 * EP + E, 2 * EP + E + D
T_BBA, T_BBB = T_BA + 3, T_BA + 5
T_COLS = T_BA + 7


def _build():
    nc = bacc.Bacc(target_bir_lowering=False)

    def dp(name, shape, dtype, isOutput=False):
        return nc.declare_dram_parameter(name, shape, dtype, isOutput)

    blobA_d = dp("blobA", [128, A_COLS], F32)
    blobB_d = dp("blobB", [128, B_COLS], F32)
    wiq_d = dp("blobC", [128, W4P_COLS], F32)
    wik_d = dp("blobD", [128, W4P_COLS], F32)
    wiv_d = dp("blobE", [128, W4_COLS], F32)
    blobF_d = dp("blobF", [128, F_COLS], F32)
    blobS_d = dp("blobS", [16, S_COLS], F32)
    blobS2_d = dp("blobS2", [16, S2_COLS], F32)
    blobT_d = dp("blobT", [64, T_COLS], F32)
    st_d = dp("state", [N, 2], I32)
    biv_d = dp("biv_r", [1, E], F32)
    bo_d = dp("bo_r", [1, E], F32)
    bva_d = dp("bva", [1, 6], F32)
    out_d = dp("out", [SH, ACT], F32, isOutput=True)

    with TileContext(nc) as tc:
        with contextlib.ExitStack() as ctx:
            wp = ctx.enter_context(tc.tile_pool(name="wp", bufs=1))
            pp = ctx.enter_context(tc.tile_pool(name="pp", bufs=7, space="PSUM"))

            def wt(shape, tag, dtype=F32):
                return wp.tile(shape, dtype, tag=tag, name=tag)

            def ps(shape):
                return pp.tile(shape, F32, tag="mm", name="mm")

            dma = nc.sync.dma_start
            gdma = nc.gpsimd.dma_start

            # ---------- blob DMAs, consumer order ----------
            # SP (HWDGE): critical-path weights in dependency order.
            blobA = wt([128, A_COLS], "blobA", F32R)
            dma(out=blobA, in_=blobA_d[:, :].bitcast(F32R))
            blobS = wt([16, S_COLS], "blobS", F32R)
            nc.scalar.dma_start(out=blobS, in_=blobS_d[:, :].bitcast(F32R))
            blobB = wt([128, B_COLS], "blobB", F32R)
            dma(out=blobB, in_=blobB_d[:, :].bitcast(F32R))
            blobT = wt([64, T_COLS], "blobT", F32R)
            nc.scalar.dma_start(out=blobT, in_=blobT_d[:, :].bitcast(F32R))
            wiqB = wt([128, W4P_COLS], "wiqB", F32R)
            dma(out=wiqB[:, 0:2 * EP], in_=wiq_d[:, 0:2 * EP].bitcast(F32R))
            dma(out=wiqB[:, 2 * EP:4 * EP], in_=wiq_d[:, 2 * EP:4 * EP].bitcast(F32R))
            wikB = wt([128, W4P_COLS], "wikB", F32R)
            dma(out=wikB[:, 0:2 * EP], in_=wik_d[:, 0:2 * EP].bitcast(F32R))
            dma(out=wikB[:, 2 * EP:4 * EP], in_=wik_d[:, 2 * EP:4 * EP].bitcast(F32R))
            wivB = wt([128, W4_COLS], "wivB", F32R)
            nc.scalar.dma_start(out=wivB, in_=wiv_d[:, :].bitcast(F32R))
            biv_bc = wt([128, E], "bivbc")
            gdma(out=biv_bc, in_=bass.AP(tensor=biv_d.ap().tensor, offset=0,
                                         ap=[[0, 128], [1, E]]))
            blobF = wt([128, F_COLS], "blobF", F32R)
            nc.scalar.dma_start(out=blobF[:, 0:F_WO],
                                in_=blobF_d[:, 0:F_WO].bitcast(F32R))
            dma(out=blobF[:, F_WO:F_COLS],
                in_=blobF_d[:, F_WO:F_COLS].bitcast(F32R))
            # Pool (SWDGE, independent of HWDGE): small / late loads.
            stfx = wt([1, 256], "stfx", I32)
            gdma(out=stfx, in_=bass.AP(tensor=st_d.ap().tensor, offset=0,
                                       ap=[[1, 1], [2, 256]]))
            stfy = wt([1, 256], "stfy", I32)
            gdma(out=stfy, in_=bass.AP(tensor=st_d.ap().tensor, offset=1,
                                       ap=[[1, 1], [2, 256]]))
            blobS2 = wt([16, S2_COLS], "blobS2", F32R)
            bo_bc = wt([SH, E], "bobc")
            bva_bc = wt([SH, 6], "bvabc")

            # ---------- tile views ----------
            st_i = [blobA[:, A_ST0:A_ST0 + 2].bitcast(F32),
                    blobA[:, A_ST1:A_ST1 + 2].bitcast(F32)]
            hT = blobA[:, A_HT:A_HT + N]
            aT = blobA[:, A_AT:A_AT + N]
            wenc = blobA[:, A_WENC:A_WENC + 16]
            id128 = blobA[:, A_ID:A_ID + 128].bitcast(F32)
            benc = blobS[:, S_BENC:S_BENC + 1].bitcast(F32)
            wqkv = [[blobB[:, B_WQKV + w * E:B_WQKV + (w + 1) * E],
                     blobS[:, S_WQKV + w * E:S_WQKV + (w + 1) * E]]
                    for w in range(3)]
            bq_t = [blobB[:, B_BA + ci * 3:B_BA + ci * 3 + 1].bitcast(F32)
                    for ci in range(4)] + [blobT[:, T_BA:T_BA + 1].bitcast(F32)]
            bk_t = [blobB[:, B_BA + ci * 3 + 1:B_BA + ci * 3 + 2].bitcast(F32)
                    for ci in range(4)] + [blobT[:, T_BA + 1:T_BA + 2].bitcast(F32)]
            bv_t = [blobB[:, B_BA + ci * 3 + 2:B_BA + ci * 3 + 3].bitcast(F32)
                    for ci in range(4)] + [blobT[:, T_BA + 2:T_BA + 3].bitcast(F32)]
            biq_t = [blobB[:, B_BB + 2 * h:B_BB + 2 * h + 1].bitcast(F32)
                     for h in range(NH)] + \
                    [blobT[:, T_BBA:T_BBA + 1].bitcast(F32),
                     blobT[:, T_BBB:T_BBB + 1].bitcast(F32)]
            bik_t = [blobB[:, B_BB + 2 * h + 1:B_BB + 2 * h + 2].bitcast(F32)
                     for h in range(NH)] + \
                    [blobT[:, T_BBA + 1:T_BBA + 2].bitcast(F32),
                     blobT[:, T_BBB + 1:T_BBB + 2].bitcast(F32)]
            wiq_t = [wiqB[:, ci * EP:(ci + 1) * EP] for ci in range(4)] + \
                    [blobT[:, T_WIQ:T_WIQ + EP]]
            wik_t = [wikB[:, ci * EP:(ci + 1) * EP] for ci in range(4)] + \
                    [blobT[:, T_WIK:T_WIK + EP]]
            wiv_t = [wivB[:, ci * E:(ci + 1) * E] for ci in range(4)] + \
                    [blobT[:, T_WIV:T_WIV + E]]
            wo8_t = []
            for h in range(4):
                wo8_t.append(blobF[:, F_WO8 + h * E:F_WO8 + (h + 1) * E])
                wo8_t.append(blobS2[:, h * E:(h + 1) * E])
            wO_t = [blobF[:, F_WO + ci * D:F_WO + (ci + 1) * D] for ci in range(4)] + \
                   [blobT[:, T_WO:T_WO + D]]
            sel_t = [blobF[:, F_SEL:F_SEL + SH].bitcast(F32),
                     blobF[:, F_SEL + SH:F_SEL + 2 * SH].bitcast(F32)]
            wva_t = [blobF[:, F_WVA:F_WVA + 6], blobS[:, S_WVA:S_WVA + 6]]

            # ---------- mask from state (gpsimd; DVE stays free) ----------
            xi = st_i
            stfxf = wt([1, 256], "stfxf")
            stfyf = wt([1, 256], "stfyf")
            nc.gpsimd.tensor_copy(out=stfxf, in_=stfx)
            nc.gpsimd.tensor_copy(out=stfyf, in_=stfy)
            xjf = wt([128, 256], "xjf")
            yjf = wt([128, 256], "yjf")
            nc.gpsimd.partition_broadcast(xjf, stfxf)
            nc.gpsimd.partition_broadcast(yjf, stfyf)

            mask_t = []
            for c in range(2):
                bx = wt([128, 256], f"bx{c}")
                by = wt([128, 256], f"by{c}")
                bx2 = wt([128, 256], f"bx2{c}")
                by2 = wt([128, 256], f"by2{c}")
                nc.gpsimd.tensor_scalar(bx, xjf, xi[c][:, 0:1], None,
                                        mybir.AluOpType.subtract)
                nc.gpsimd.tensor_scalar(by, yjf, xi[c][:, 1:2], None,
                                        mybir.AluOpType.subtract)
                nc.gpsimd.tensor_scalar(bx2, bx, -4.0, None, mybir.AluOpType.is_ge)
                nc.gpsimd.tensor_scalar(bx, bx, 4.0, None, mybir.AluOpType.is_le)
                nc.gpsimd.tensor_scalar(by2, by, -2.0, None, mybir.AluOpType.is_ge)
                nc.gpsimd.tensor_scalar(by, by, 2.0, None, mybir.AluOpType.is_le)
                nc.gpsimd.tensor_tensor(bx, bx, bx2, mybir.AluOpType.mult)
                nc.gpsimd.tensor_tensor(by, by, by2, mybir.AluOpType.mult)
                prox = wt([128, 256], f"prox{c}")
                nc.gpsimd.tensor_tensor(prox, bx, by, mybir.AluOpType.mult)
                mk = wt([128, 256], f"mask{c}")
                nc.gpsimd.affine_select(out=mk, in_=prox, pattern=[[1, 256]],
                                        compare_op=mybir.AluOpType.is_gt,
                                        fill=0.0, base=-c * 128,
                                        channel_multiplier=-1)
                mask_t.append(mk)

            # ---------- obs^T = W_enc^T @ hidden^T + b_enc ----------
            obsT = wt([16, N], "obsT", F32R)
            p0 = ps([16, N])
            nc.tensor.matmul(p0, wenc, hT, start=True, stop=True)
            nc.vector.tensor_scalar(obsT, p0, benc, None, mybir.AluOpType.add)
            CT = [aT, obsT]

            # ---------- t^T = W^T C'^T + b (x3) ----------
            def proj_t(wtiles, btiles, tag):
                outs = []
                for mi, (ms, ml) in enumerate(EC):
                    p = ps([ml, N])
                    for ci in range(2):
                        nc.tensor.matmul(p, wtiles[ci][:, ms:ms + ml], CT[ci],
                                         start=(ci == 0), stop=(ci == 1))
                    t = wt([ml, N], f"{tag}{mi}", F32R)
                    nc.vector.tensor_scalar(t, p, btiles[mi], None,
                                            mybir.AluOpType.add)
                    outs.append(t)
                return outs

            tqT = proj_t(wqkv[0], bq_t, "tqT")
            tkT = proj_t(wqkv[1], bk_t, "tkT")
            tvT = proj_t(wqkv[2], bv_t, "tvT")

            # ---------- q^T / k^T (HC-tiled so head rows start at part 0) ----
            def proj_qk(wi_t, tT, bt, tag, eng):
                outs = []
                for mi, (ms, ml) in enumerate(QKM):
                    p = ps([ml, N])
                    for ci, (cs, cl) in enumerate(EC):
                        nc.tensor.matmul(p, wi_t[ci][:, ms:ms + ml], tT[ci],
                                         start=(ci == 0), stop=(ci == len(EC) - 1))
                    t = wt([ml, N], f"{tag}{mi}", F32R)
                    if eng == "act":
                        nc.scalar.activation(t, p,
                                             mybir.ActivationFunctionType.Identity,
                                             bias=bt[mi], scale=1.0)
                    else:
                        nc.vector.tensor_scalar(t, p, bt[mi], None,
                                                mybir.AluOpType.add)
                    outs.append(t)
                return outs

            qT = proj_qk(wiq_t, tqT, biq_t, "qT", "dve")
            kT = proj_qk(wik_t, tkT, bik_t, "kT", "dve")

            # ---------- S_h, S_h^T -> E_h, E_h^T ----------
            E_t = [[None, None] for _ in range(NH)]
            ET_t = [[None, None] for _ in range(NH)]
            for h in range(NH):
                hs = [(h, 0, 128), (4 + h // 2, 32 * (h % 2), 32)]
                for mj in range(2):
                    pS = ps([128, N])
                    pST = ps([128, N])
                    for ci, (ti, rs, rl) in enumerate(hs):
                        st_, sp = (ci == 0), (ci == len(hs) - 1)
                        nc.tensor.matmul(
                            pS, qT[ti][rs:rs + rl, mj * 128:(mj + 1) * 128],
                            kT[ti][rs:rs + rl, :], start=st_, stop=sp)
                        nc.tensor.matmul(
                            pST, kT[ti][rs:rs + rl, mj * 128:(mj + 1) * 128],
                            qT[ti][rs:rs + rl, :], start=st_, stop=sp)
                    Eh = wt([128, N], f"E{h}_{mj}", F32R)
                    ETh = wt([128, N], f"ET{h}_{mj}", F32R)
                    nc.scalar.activation(Eh, pS, mybir.ActivationFunctionType.Exp,
                                         scale=SCALE)
                    nc.scalar.activation(ETh, pST,
                                         mybir.ActivationFunctionType.Exp,
                                         scale=SCALE)
                    E_t[h][mj] = Eh
                    ET_t[h][mj] = ETh

            gdma(out=blobS2, in_=blobS2_d[:, :].bitcast(F32R))
            gdma(out=bo_bc, in_=bass.AP(tensor=bo_d.ap().tensor, offset=0,
                                        ap=[[0, SH], [1, E]]))
            gdma(out=bva_bc, in_=bass.AP(tensor=bva_d.ap().tensor, offset=0,
                                         ap=[[0, SH], [1, 6]]))
            # ---------- v = (t_v^T)^T Wiv + biv  [n, E] ----------
            v_t = []
            for nt in range(2):
                vt = wt([128, E], f"v{nt}", F32R)
                for ns, nl in ((0, 288), (288, 288)):
                    p = ps([128, nl])
                    for ci, (cs, cl) in enumerate(EC):
                        nc.tensor.matmul(
                            p, tvT[ci][:, nt * 128:(nt + 1) * 128],
                            wiv_t[ci][:, ns:ns + nl],
                            start=(ci == 0), stop=(ci == len(EC) - 1))
                    nc.vector.tensor_tensor(vt[:, ns:ns + nl], p,
                                            biv_bc[:, ns:ns + nl],
                                            mybir.AluOpType.add)
                v_t.append(vt)

            # ---------- phase B: this core's 32 agents (agents on free dim) --
            mcT = []
            for km in range(2):
                p = ps([128, SH])
                for c in range(2):
                    nc.tensor.matmul(
                        p, mask_t[c][:, km * 128:(km + 1) * 128],
                        sel_t[c], start=(c == 0), stop=(c == 1))
                t = wt([128, SH], f"mcT{km}", F32R)
                nc.any.tensor_copy(out=t, in_=p)
                mcT.append(t)
            ones_t = wt([128, 1], "ones_t")
            nc.vector.memset(ones_t, 1.0)
            pn = ps([SH, 1])
            for c in range(2):
                nc.tensor.matmul(pn, mcT[c].bitcast(F32), ones_t,
                                 start=(c == 0), stop=(c == 1))
            n_i = wt([SH, 1], "n_i")
            nc.any.tensor_copy(out=n_i, in_=pn)

            ctxT8 = [None] * 8
            for hg in (0, 2):
                RTg, WTg = {}, {}
                for h in (hg, hg + 1):
                    for jm in range(2):
                        p = ps([128, SH])
                        for kc in range(2):
                            nc.tensor.matmul(
                                p, ET_t[h][kc][:, jm * 128:(jm + 1) * 128],
                                mcT[kc], start=(kc == 0), stop=(kc == 1))
                        rt = wt([128, SH], f"RT{h}_{jm}", F32R)
                        nc.vector.tensor_scalar(rt, p, 1e-9, None,
                                                mybir.AluOpType.max)
                        with nc.allow_low_precision(reason="fp32r attn renorm"):
                            nc.vector.reciprocal(rt, rt)
                        nc.vector.tensor_tensor(rt, rt, mcT[jm].bitcast(F32),
                                                mybir.AluOpType.mult)
                        RTg[(h, jm)] = rt
                for h in (hg, hg + 1):
                    for km in range(2):
                        p = ps([128, SH])
                        for jc in range(2):
                            nc.tensor.matmul(
                                p, E_t[h][jc][:, km * 128:(km + 1) * 128],
                                RTg[(h, jc)], start=(jc == 0), stop=(jc == 1))
                        wtl = wt([128, SH], f"WT{h}_{km}", F32R)
                        nc.vector.tensor_tensor(wtl, p, mcT[km].bitcast(F32),
                                                mybir.AluOpType.mult)
                        WTg[(h, km)] = wtl
                for h in (hg, hg + 1):
                    for dm, (ds, dl) in enumerate([(0, 128), (128, 16)]):
                        p = ps([dl, SH])
                        for kc in range(2):
                            nc.tensor.matmul(
                                p, v_t[kc][:, HD * h + ds:HD * h + ds + dl],
                                WTg[(h, kc)], start=(kc == 0), stop=(kc == 1))
                        t = wt([dl, SH], f"cT{2 * h + dm}", F32R)
                        nc.any.tensor_copy(out=t, in_=p)
                        ctxT8[2 * h + dm] = t

            # ho = ctx @ Wo + n_i * bo   [32, 576]
            ho_sb = wt([SH, E], "hosb")
            for ns, nl in ((0, 288), (288, 288)):
                p = ps([SH, nl])
                for ci in range(8):
                    nc.tensor.matmul(p, ctxT8[ci], wo8_t[ci][:, ns:ns + nl],
                                     start=(ci == 0), stop=(ci == 7))
                nc.vector.scalar_tensor_tensor(
                    out=ho_sb[:, ns:ns + nl], in0=bo_bc[:, ns:ns + nl],
                    scalar=n_i, in1=p,
                    op0=mybir.AluOpType.mult, op1=mybir.AluOpType.add)

            # ho^T via PE transpose; hfeat^T = W_O^T ho^T; VA; dueling Q
            hoT = []
            for ci, (cs, cl) in enumerate(EC):
                p = ps([cl, SH])
                nc.tensor.transpose(p, ho_sb[:, cs:cs + cl], id128[0:SH, 0:SH])
                t = wt([cl, SH], f"hoT{ci}", F32R)
                nc.any.tensor_copy(out=t, in_=p)
                hoT.append(t)
            hfT = []
            for mi, (ms, ml) in enumerate(DC):
                p = ps([ml, SH])
                for ci, (cs, cl) in enumerate(EC):
                    nc.tensor.matmul(p, wO_t[ci][:, ms:ms + ml], hoT[ci],
                                     start=(ci == 0), stop=(ci == len(EC) - 1))
                t = wt([ml, SH], f"hfT{mi}", F32R)
                nc.any.tensor_copy(out=t, in_=p)
                hfT.append(t)
            pVA = ps([SH, 6])
            for ci in range(2):
                nc.tensor.matmul(pVA, hfT[ci], wva_t[ci],
                                 start=(ci == 0), stop=(ci == 1))
            VA = wt([SH, 6], "VA")
            nc.vector.tensor_tensor(VA, pVA, bva_bc, mybir.AluOpType.add)
            sA = wt([SH, 1], "sA")
            nc.vector.reduce_sum(sA, VA[:, 1:6], axis=mybir.AxisListType.X)
            vm = wt([SH, 1], "vm")
            nc.vector.scalar_tensor_tensor(out=vm, in0=sA, scalar=-0.2,
                                           in1=VA[:, 0:1],
                                           op0=mybir.AluOpType.mult,
                                           op1=mybir.AluOpType.add)
            Q_sb = wt([SH, ACT], "Qsb")
            nc.vector.tensor_scalar(Q_sb, VA[:, 1:6], vm, None,
                                    mybir.AluOpType.add)
            nc.gpsimd.dma_start(out=out_d[:, :], in_=Q_sb)

    nc.compile()
    return nc


_NC_CACHE = {}


def _make_in_maps(inputs):
    f32 = np.float32
    g = lambda k: np.ascontiguousarray(np.asarray(inputs[k]), dtype=f32)
    perm = lambda w: np.concatenate([w[16:144], w[0:16]], axis=0)

    hidden, action = g("hidden_state_n"), g("action_n")
    state = np.ascontiguousarray(np.asarray(inputs["state_n"]), dtype=np.int32)
    Wq, Wk, Wv = perm(g("Wq")), perm(g("Wk")), perm(g("Wv"))
    biasA = np.stack([g("bq"), g("bk"), g("bv")], axis=1)          # [576,3]
    biasB = np.stack([g("biq"), g("bik")], axis=1)                 # [576,2]
    Wiq, Wik, Wiv = g("Wiq"), g("Wik"), g("Wiv")

    def padqk(w):  # [X,576] -> [X,640]: 4 head-mains + 2 padded tail blocks
        mains = [w[:, 144 * h:144 * h + 128] for h in range(4)]
        z = np.zeros((w.shape[0], 16), f32)
        tails = [np.concatenate([w[:, 144 * h + 128:144 * h + 144], z,
                                 w[:, 144 * (h + 1) + 128:144 * (h + 1) + 144],
                                 z], axis=1) for h in (0, 2)]
        return np.concatenate(mains + tails, axis=1)

    WiqP, WikP = padqk(Wiq), padqk(Wik)
    # biasB mains [144h:144h+128] -> rows 128h..; tails padded like padqk
    bbz = np.zeros((16, 2), f32)
    bbTA = np.concatenate([biasB[128:144], bbz, biasB[272:288], bbz], axis=0)
    bbTB = np.concatenate([biasB[416:432], bbz, biasB[560:576], bbz], axis=0)
    Wo, W_O = g("Wo_proj"), g("W_O")
    Wva = np.concatenate([g("W_val").reshape(D, 1),
                          g("W_adv").reshape(D, ACT)], axis=1)     # [144,6]
    eye = np.eye(N, dtype=f32)

    def blocks128(w):   # rows 0:512 -> [128, 4*cols]
        return w[0:512].reshape(4, 128, -1).transpose(1, 0, 2).reshape(128, -1)

    def hblocks(w, rows, r0):  # 144-row blocks -> [rows, 4*cols]
        return np.concatenate([w[144 * h + r0:144 * h + r0 + rows]
                               for h in range(4)], axis=1)

    state_f = state.astype(f32)
    blobA = np.concatenate([
        state_f[0:128], state_f[128:256],
        np.ascontiguousarray(hidden.T), np.ascontiguousarray(action.T),
        g("W_enc"), np.eye(128, dtype=f32)], axis=1)
    blobB = np.concatenate([
        np.concatenate([Wq[0:128], Wk[0:128], Wv[0:128]], axis=1),
        blocks128(biasA), hblocks(biasB, 128, 0)], axis=1)
    blobF = np.concatenate([
        hblocks(Wo, 128, 0), blocks128(W_O),
        np.zeros((128, 2 * SH), f32), Wva[0:128]], axis=1)
    blobS = np.concatenate([
        np.concatenate([Wq[128:144], Wk[128:144], Wv[128:144]], axis=1),
        hblocks(biasB, 16, 128),
        Wva[128:144], g("b_enc").reshape(16, 1)], axis=1)
    blobS2 = hblocks(Wo, 16, 128)
    blobT = np.concatenate([
        WiqP[512:576], WikP[512:576], Wiv[512:576], W_O[512:576],
        biasA[512:576], bbTA, bbTB], axis=1)

    shared = {
        "blobA": np.ascontiguousarray(blobA, dtype=f32),
        "blobB": np.ascontiguousarray(blobB, dtype=f32),
        "blobC": np.ascontiguousarray(blocks128(WiqP), dtype=f32),
        "blobD": np.ascontiguousarray(blocks128(WikP), dtype=f32),
        "blobE": np.ascontiguousarray(blocks128(Wiv), dtype=f32),
        "blobS": np.ascontiguousarray(blobS, dtype=f32),
        "blobS2": np.ascontiguousarray(blobS2, dtype=f32),
        "blobT": np.ascontiguousarray(blobT, dtype=f32),
        "state": state,
        "biv_r": g("biv").reshape(1, E),
        "bo_r": g("bo_proj").reshape(1, E),
        "bva": np.concatenate([g("b_val").reshape(1),
                               g("b_adv").reshape(ACT)]).reshape(1, 6)
        .astype(f32),
    }
    in_maps = []
    for c in range(NCORES):
        sel = eye[:, c * SH:(c + 1) * SH]              # [256, 32]
        selpack = np.concatenate([sel[0:128], sel[128:256]], axis=1)  # [128,64]
        bF = blobF.copy()
        bF[:, F_SEL:F_SEL + 2 * SH] = selpack
        m = dict(shared)
        m["blobF"] = np.ascontiguousarray(bF, dtype=f32)
        in_maps.append(m)
    return in_maps


def kernel(**inputs):
    if "nc" not in _NC_CACHE:
        _NC_CACHE["nc"] = _build()
    nc = _NC_CACHE["nc"]
    in_maps = _make_in_maps(inputs)
    res = bass_utils.run_bass_kernel_spmd(nc, in_maps, core_ids=list(range(NCORES)))
    return np.concatenate([res.results[c]["out"] for c in range(NCORES)], axis=0)



# revision 2
# speedup vs baseline: 2.7868x; 2.7868x over previous
"""AttentionCritic Trainium2 kernel — 8-core SPMD, no collectives.

Math restructuring (exact up to fp assoc.):
  mask[i,j] = (|x_i-x_j|<=4)&(|y_i-y_j|<=2)&(j>i)
  C = [obs, action];  obs = h@W_enc + b_enc
  q = C@Wq'@Wiq + ... = action@WqA + hidden@Wqh + bqf   (host-fused weights)
  S_h = q_h k_h^T / 12  (shared over agents); E_h = exp(S_h)
  D[i,h,j] = sum_k E_h[j,k] mask[i,k];  R = mask/max(D,1e-9)
  W[i,h,k] = mask[i,k] * sum_j R[i,h,j] E_h[j,k]
  ctx0[i,h] = sum_k W[i,h,k] v0_h[k]    (v0 = C@Wvf, bias deferred)
  VA[i]   = sum_h ctx0[i,h] @ Wbig_h + n_i*nvec + bva
            where Wbig = Wo_proj@W_O@[W_val|W_adv],
                  nvec = bvf@Wbig + (bo_proj@W_O)@[W_val|W_adv]
  Q = V + A - mean(A)

All matmul inputs bf16 (fp32 PSUM accumulation); biases/tail fp32.
Phase A (q/k/E/ET/v) replicated on all 8 cores; phase B data-parallel over
agents via a per-core one-hot selector (the only per-core input).
"""

import sys

for _p in ("/opt/trn_rl_repo",):
    if _p not in sys.path:
        sys.path.append(_p)

import contextlib

import numpy as np
import ml_dtypes

import concourse.bass as bass
import concourse.bacc as bacc
import concourse.mybir as mybir
from concourse.tile import TileContext
from concourse import bass_utils

N, HID, ACT, NH = 256, 128, 5, 4
D, E, HD = 144, 576, 144
NCORES = 8
SH = N // NCORES  # 32
F32 = mybir.dt.float32
BF16 = mybir.dt.bfloat16
SCALE = 1.0 / 12.0
EP = 640
QKM = [(0, 128), (128, 128), (256, 128), (384, 128), (512, 64), (576, 64)]

# blobW bf16 [128, 3712]: WqA(640) Wqh(640) WkA(640) Wkh(640) WvA(576) Wvh(576)
W_QA, W_QH, W_KA, W_KH, W_VA_, W_VH = 0, EP, 2 * EP, 3 * EP, 4 * EP, 4 * EP + E
W_COLS = 4 * EP + 2 * E
# blobF bf16 [128, 112]: selpack(64) Wbig8(48)   (per-core: sel differs)
F_SEL, F_WB = 0, 64
F_COLS = F_WB + 48
# fside f32 [128, 16]: negx0 negy0 negx1 negy1 | q bias (4 main, 2 tail) | k
FS_NEG, FS_BQ, FS_BK = 0, 4, 10
FS_COLS = 16


def _build():
    nc = bacc.Bacc(target_bir_lowering=False)

    def dp(name, shape, dtype, isOutput=False):
        return nc.declare_dram_parameter(name, shape, dtype, isOutput)

    blobA_d = dp("blobA", [128, 512], BF16)     # hT(256) aT(256)
    blobW_d = dp("blobW", [128, W_COLS], BF16)
    blobF_d = dp("blobF", [128, F_COLS], BF16)
    fside_d = dp("fside", [128, FS_COLS], F32)
    srows_d = dp("srows", [2, N], BF16)         # x row, y row
    nb2_d = dp("nb2", [1, 12], F32)             # nvec(6) bva(6)
    out_d = dp("out", [SH, ACT], F32, isOutput=True)

    with TileContext(nc) as tc:
        with contextlib.ExitStack() as ctx:
            wp = ctx.enter_context(tc.tile_pool(name="wp", bufs=1))
            pp = ctx.enter_context(tc.tile_pool(name="pp", bufs=7, space="PSUM"))

            def wt(shape, tag, dtype=BF16):
                return wp.tile(shape, dtype, tag=tag, name=tag)

            def ps(shape):
                return pp.tile(shape, F32, tag="mm", name="mm")

            dma = nc.sync.dma_start
            sdma = nc.scalar.dma_start
            gdma = nc.gpsimd.dma_start

            # ---------- DMAs, consumer order ----------
            blobA = wt([128, 512], "blobA")
            dma(out=blobA, in_=blobA_d[:, :])
            blobW = wt([128, W_COLS], "blobW")
            dma(out=blobW[:, 0:2 * EP], in_=blobW_d[:, 0:2 * EP])
            fside = wt([128, FS_COLS], "fside", F32)
            sdma(out=fside, in_=fside_d[:, :])
            sdma(out=blobW[:, 2 * EP:4 * EP], in_=blobW_d[:, 2 * EP:4 * EP])
            dma(out=blobW[:, 4 * EP:W_COLS], in_=blobW_d[:, 4 * EP:W_COLS])
            blobF = wt([128, F_COLS], "blobF")
            sdma(out=blobF, in_=blobF_d[:, :])
            # SWDGE broadcasts
            xjf = wt([128, N], "xjf")
            gdma(out=xjf, in_=bass.AP(tensor=srows_d.ap().tensor, offset=0,
                                      ap=[[0, 128], [1, N]]))
            yjf = wt([128, N], "yjf")
            gdma(out=yjf, in_=bass.AP(tensor=srows_d.ap().tensor, offset=N,
                                      ap=[[0, 128], [1, N]]))
            nb_bc = wt([SH, 12], "nbbc", F32)
            gdma(out=nb_bc, in_=bass.AP(tensor=nb2_d.ap().tensor, offset=0,
                                        ap=[[0, SH], [1, 12]]))

            # ---------- tile views ----------
            hT = blobA[:, 0:256]
            aT = blobA[:, 256:512]
            movs = [aT, hT]
            wq_t = [blobW[:, W_QA:W_QA + EP], blobW[:, W_QH:W_QH + EP]]
            wk_t = [blobW[:, W_KA:W_KA + EP], blobW[:, W_KH:W_KH + EP]]
            wv_t = [blobW[:, W_VA_:W_VA_ + E], blobW[:, W_VH:W_VH + E]]
            sel_t = [blobF[:, F_SEL:F_SEL + SH], blobF[:, F_SEL + SH:F_SEL + 2 * SH]]
            wbig_t = [blobF[:, F_WB + 6 * g:F_WB + 6 * g + 6] for g in range(8)]
            negx = [fside[:, 0:1], fside[:, 2:3]]
            negy = [fside[:, 1:2], fside[:, 3:4]]
            bq_t = [fside[:, FS_BQ + h:FS_BQ + h + 1] for h in range(4)] + \
                   [fside[0:64, FS_BQ + 4:FS_BQ + 5], fside[0:64, FS_BQ + 5:FS_BQ + 6]]
            bk_t = [fside[:, FS_BK + h:FS_BK + h + 1] for h in range(4)] + \
                   [fside[0:64, FS_BK + 4:FS_BK + 5], fside[0:64, FS_BK + 5:FS_BK + 6]]

            # ---------- mask from state (DVE only; dx^2<=16 & dy^2<=4 & j>i) --
            mask_t = []
            for c in range(2):
                bx = wt([128, N], f"bx{c}")
                by = wt([128, N], f"by{c}")
                nc.vector.tensor_scalar(bx, xjf, negx[c], None,
                                        mybir.AluOpType.add)
                nc.vector.tensor_scalar(by, yjf, negy[c], None,
                                        mybir.AluOpType.add)
                bx2 = wt([128, N], f"bx2{c}")
                by2 = wt([128, N], f"by2{c}")
                nc.vector.tensor_tensor(bx2, bx, bx, mybir.AluOpType.mult)
                nc.vector.tensor_tensor(by2, by, by, mybir.AluOpType.mult)
                nc.vector.tensor_scalar(bx, bx2, 16.0, None, mybir.AluOpType.is_le)
                nc.vector.tensor_scalar(by, by2, 4.0, None, mybir.AluOpType.is_le)
                prox = wt([128, N], f"prox{c}")
                nc.vector.tensor_tensor(prox, bx, by, mybir.AluOpType.mult)
                mk = wt([128, N], f"mask{c}")
                nc.gpsimd.affine_select(out=mk, in_=prox, pattern=[[1, N]],
                                        compare_op=mybir.AluOpType.is_gt,
                                        fill=0.0, base=-c * 128,
                                        channel_multiplier=-1)
                mask_t.append(mk)

            # ---------- qT / kT: fused projection  [e', n] ----------
            def proj(w_t, b_t, tag):
                outs = []
                for mi, (ms, ml) in enumerate(QKM):
                    p = ps([ml, N])
                    for ci in range(2):
                        nc.tensor.matmul(p, w_t[ci][:, ms:ms + ml], movs[ci],
                                         start=(ci == 0), stop=(ci == 1))
                    t = wt([ml, N], f"{tag}{mi}")
                    nc.vector.tensor_scalar(t, p, b_t[mi], None,
                                            mybir.AluOpType.add)
                    outs.append(t)
                return outs

            qT = proj(wq_t, bq_t, "qT")
            kT = proj(wk_t, bk_t, "kT")

            # ---------- S_h, S_h^T -> E_h, E_h^T (bf16) ----------
            E_t = [[None, None] for _ in range(NH)]
            ET_t = [[None, None] for _ in range(NH)]
            for h in range(NH):
                hs = [(h, 0, 128), (4 + h // 2, 32 * (h % 2), 32)]
                for mj in range(2):
                    pS = ps([128, N])
                    pST = ps([128, N])
                    for ci, (ti, rs, rl) in enumerate(hs):
                        st_, sp = (ci == 0), (ci == len(hs) - 1)
                        nc.tensor.matmul(
                            pS, qT[ti][rs:rs + rl, mj * 128:(mj + 1) * 128],
                            kT[ti][rs:rs + rl, :], start=st_, stop=sp)
                        nc.tensor.matmul(
                            pST, kT[ti][rs:rs + rl, mj * 128:(mj + 1) * 128],
                            qT[ti][rs:rs + rl, :], start=st_, stop=sp)
                    Eh = wt([128, N], f"E{h}_{mj}")
                    ETh = wt([128, N], f"ET{h}_{mj}")
                    nc.scalar.activation(Eh, pS, mybir.ActivationFunctionType.Exp,
                                         scale=SCALE)
                    nc.scalar.activation(ETh, pST,
                                         mybir.ActivationFunctionType.Exp,
                                         scale=SCALE)
                    E_t[h][mj] = Eh
                    ET_t[h][mj] = ETh

            # ---------- v0 = C @ Wvf  [n, E] (no bias; folded into nvec) ----
            v_t = []
            for nt in range(2):
                vt = wt([128, E], f"v{nt}")
                for ns, nl in ((0, 288), (288, 288)):
                    p = ps([128, nl])
                    for ci in range(2):
                        nc.tensor.matmul(
                            p, movs[ci][:, nt * 128:(nt + 1) * 128],
                            wv_t[ci][:, ns:ns + nl],
                            start=(ci == 0), stop=(ci == 1))
                    nc.any.tensor_copy(out=vt[:, ns:ns + nl], in_=p)
                v_t.append(vt)

            # ---------- phase B: this core's 32 agents (agents on free dim) --
            mcT = []
            for km in range(2):
                p = ps([128, SH])
                for c in range(2):
                    nc.tensor.matmul(
                        p, mask_t[c][:, km * 128:(km + 1) * 128],
                        sel_t[c], start=(c == 0), stop=(c == 1))
                t = wt([128, SH], f"mcT{km}")
                nc.any.tensor_copy(out=t, in_=p)
                mcT.append(t)
            ones_t = wt([128, 1], "ones_t")
            nc.vector.memset(ones_t, 1.0)
            pn = ps([SH, 1])
            for c in range(2):
                nc.tensor.matmul(pn, mcT[c], ones_t,
                                 start=(c == 0), stop=(c == 1))
            n_i = wt([SH, 1], "n_i", F32)
            nc.any.tensor_copy(out=n_i, in_=pn)

            RT = {}
            for h in range(NH):
                for jm in range(2):
                    p = ps([128, SH])
                    for kc in range(2):
                        nc.tensor.matmul(
                            p, ET_t[h][kc][:, jm * 128:(jm + 1) * 128],
                            mcT[kc], start=(kc == 0), stop=(kc == 1))
                    rtf = wt([128, SH], f"RTf{h}_{jm}", F32)
                    nc.vector.tensor_scalar(rtf, p, 1e-9, None,
                                            mybir.AluOpType.max)
                    with nc.allow_low_precision(reason="attn renorm"):
                        nc.vector.reciprocal(rtf, rtf)
                    rt = wt([128, SH], f"RT{h}_{jm}")
                    nc.vector.tensor_tensor(rt, rtf, mcT[jm],
                                            mybir.AluOpType.mult)
                    RT[(h, jm)] = rt
            WT = {}
            for h in range(NH):
                for km in range(2):
                    p = ps([128, SH])
                    for jc in range(2):
                        nc.tensor.matmul(
                            p, E_t[h][jc][:, km * 128:(km + 1) * 128],
                            RT[(h, jc)], start=(jc == 0), stop=(jc == 1))
                    wtl = wt([128, SH], f"WT{h}_{km}")
                    nc.vector.tensor_tensor(wtl, p, mcT[km],
                                            mybir.AluOpType.mult)
                    WT[(h, km)] = wtl
            ctxT8 = [None] * 8
            for h in range(NH):
                for dm, (ds, dl) in enumerate([(0, 128), (128, 16)]):
                    p = ps([dl, SH])
                    for kc in range(2):
                        nc.tensor.matmul(
                            p, v_t[kc][:, HD * h + ds:HD * h + ds + dl],
                            WT[(h, kc)], start=(kc == 0), stop=(kc == 1))
                    t = wt([dl, SH], f"cT{2 * h + dm}")
                    nc.any.tensor_copy(out=t, in_=p)
                    ctxT8[2 * h + dm] = t

            # ---------- VA = sum_g ctxT8[g]^T @ Wbig8[g] + n_i*nvec + bva ----
            pVA = ps([SH, 6])
            for g in range(8):
                dl = 128 if g % 2 == 0 else 16
                nc.tensor.matmul(pVA, ctxT8[g], wbig_t[g][0:dl, :],
                                 start=(g == 0), stop=(g == 7))
            VAt = wt([SH, 6], "VAt", F32)
            nc.vector.scalar_tensor_tensor(
                out=VAt, in0=nb_bc[:, 0:6], scalar=n_i, in1=pVA,
                op0=mybir.AluOpType.mult, op1=mybir.AluOpType.add)
            VA = wt([SH, 6], "VA", F32)
            nc.vector.tensor_tensor(VA, VAt, nb_bc[:, 6:12],
                                    mybir.AluOpType.add)
            sA = wt([SH, 1], "sA", F32)
            nc.vector.reduce_sum(sA, VA[:, 1:6], axis=mybir.AxisListType.X)
            vm = wt([SH, 1], "vm", F32)
            nc.vector.scalar_tensor_tensor(out=vm, in0=sA, scalar=-0.2,
                                           in1=VA[:, 0:1],
                                           op0=mybir.AluOpType.mult,
                                           op1=mybir.AluOpType.add)
            Q_sb = wt([SH, ACT], "Qsb", F32)
            nc.vector.tensor_scalar(Q_sb, VA[:, 1:6], vm, None,
                                    mybir.AluOpType.add)
            nc.gpsimd.dma_start(out=out_d[:, :], in_=Q_sb)

    nc.compile()
    return nc


_NC_CACHE = {}
BF = ml_dtypes.bfloat16


def _make_in_maps(inputs):
    f32 = np.float32
    g = lambda k: np.asarray(inputs[k], dtype=f32)

    hidden, action = g("hidden_state_n"), g("action_n")
    state = np.asarray(inputs["state_n"]).astype(np.int32)
    W_enc, b_enc = g("W_enc"), g("b_enc")
    Wiq, Wik, Wiv = g("Wiq"), g("Wik"), g("Wiv")

    # host-fused projection weights: C=[obs(16), action(128)]
    def fuse(Wo_, bo_, Wi_, bi_):
        Wf = Wo_ @ Wi_                              # [144, 576]
        WA = Wf[16:144]                             # action rows [128, 576]
        Wh = W_enc @ Wf[0:16]                       # hidden rows [128, 576]
        bf = b_enc @ Wf[0:16] + bo_ @ Wi_ + bi_     # [576]
        return WA, Wh, bf

    WqA, Wqh, bqf = fuse(g("Wq"), g("bq"), Wiq, g("biq"))
    WkA, Wkh, bkf = fuse(g("Wk"), g("bk"), Wik, g("bik"))
    WvA, Wvh, bvf = fuse(g("Wv"), g("bv"), Wiv, g("biv"))

    # fused output chain: Wbig [576, 6], nvec [6]
    Wva6 = np.concatenate([g("W_val").reshape(D, 1),
                           g("W_adv").reshape(D, ACT)], axis=1)    # [144,6]
    WoWO = g("Wo_proj") @ g("W_O")                                 # [576,144]
    Wbig = WoWO @ Wva6                                             # [576,6]
    nvec = bvf @ Wbig + (g("bo_proj") @ g("W_O")) @ Wva6           # [6]
    bva6 = np.concatenate([g("b_val").reshape(1), g("b_adv")])     # [6]

    def padqk(w):  # [128,576] -> [128,640]: 4 head-mains + 2 padded tails
        mains = [w[:, 144 * h:144 * h + 128] for h in range(4)]
        z = np.zeros((w.shape[0], 16), f32)
        tails = [np.concatenate([w[:, 144 * h + 128:144 * h + 144], z,
                                 w[:, 144 * (h + 1) + 128:144 * (h + 1) + 144],
                                 z], axis=1) for h in (0, 2)]
        return np.concatenate(mains + tails, axis=1)

    def bias_cols(b):  # [576] -> [128, 6] per-QKM-tile scalar columns
        cols = np.zeros((128, 6), f32)
        for h in range(4):
            cols[:, h] = b[144 * h:144 * h + 128]
        for t, h in enumerate((0, 2)):
            cols[0:16, 4 + t] = b[144 * h + 128:144 * h + 144]
            cols[32:48, 4 + t] = b[144 * (h + 1) + 128:144 * (h + 1) + 144]
        return cols

    blobA = np.concatenate([np.ascontiguousarray(hidden.T),
                            np.ascontiguousarray(action.T)], axis=1)
    blobW = np.concatenate([padqk(WqA), padqk(Wqh), padqk(WkA), padqk(Wkh),
                            WvA, Wvh], axis=1)
    state_f = state.astype(f32)
    fside = np.zeros((128, FS_COLS), f32)
    fside[:, 0] = -state_f[0:128, 0]
    fside[:, 1] = -state_f[0:128, 1]
    fside[:, 2] = -state_f[128:256, 0]
    fside[:, 3] = -state_f[128:256, 1]
    fside[:, FS_BQ:FS_BQ + 6] = bias_cols(bqf)
    fside[:, FS_BK:FS_BK + 6] = bias_cols(bkf)
    srows = np.ascontiguousarray(state_f.T)                        # [2, 256]
    nb2 = np.concatenate([nvec, bva6]).reshape(1, 12).astype(f32)

    wbig8 = np.zeros((128, 48), f32)
    for h in range(4):
        wbig8[:, 12 * h:12 * h + 6] = Wbig[144 * h:144 * h + 128]
        wbig8[0:16, 12 * h + 6:12 * h + 12] = Wbig[144 * h + 128:144 * (h + 1)]

    eye = np.eye(N, dtype=f32)
    shared = {
        "blobA": blobA.astype(BF),
        "blobW": blobW.astype(BF),
        "fside": fside,
        "srows": srows.astype(BF),
        "nb2": nb2,
    }
    in_maps = []
    for c in range(NCORES):
        sel = eye[:, c * SH:(c + 1) * SH]              # [256, 32]
        selpack = np.concatenate([sel[0:128], sel[128:256]], axis=1)  # [128,64]
        bF = np.concatenate([selpack, wbig8], axis=1)
        m = dict(shared)
        m["blobF"] = np.ascontiguousarray(bF).astype(BF)
        in_maps.append(m)
    return in_maps


def kernel(**inputs):
    if "nc" not in _NC_CACHE:
        _NC_CACHE["nc"] = _build()
    nc = _NC_CACHE["nc"]
    in_maps = _make_in_maps(inputs)
    res = bass_utils.run_bass_kernel_spmd(nc, in_maps, core_ids=list(range(NCORES)))
    return np.concatenate([res.results[c]["out"] for c in range(NCORES)], axis=0)


# revision 10
# speedup vs baseline: 3.0750x; 1.1034x over previous
"""AttentionCritic Trainium2 kernel — 8-core SPMD, no collectives.

Fast path (zero q/k biases, which setup_inputs produces):
  S_h = q_h k_h^T / 12 = C Mh C^T / 12,  Mh = (Wq Wiq)_h (Wk Wik)_h^T  (host)
  C = [obs, action], obs folded via W_enc into the u-projection weights:
  uT = Mall^T C^T computed as  WuA^T aT + Wuh^T hT   (no k projection at all)
  E_h = exp(S_h/12);  D[i,h,j] = sum_k E_h[j,k] m[i,k];  R = m/max(D,1e-9)
  W[i,h,k] = m[i,k] * sum_j R[i,h,j] E_h[j,k];  ctx0[i,h] = sum_k W v0_h[k]
  VA = sum_h ctx0_h @ Wbig_h + n_i*nvec + bva   (rank-1 PE updates)
  Q = V + A - mean(A)

mask computed directly in transposed [j, i-local] form on DVE from per-core
broadcast rows (x, y, global index) and per-partition j coords; n_i via PE.
All matmul inputs bf16 (fp32 PSUM); DMAs split over 4 HWDGE queues.
General path (nonzero q/k biases): separate q/k projections, same phase B.
"""

import sys

for _p in ("/opt/trn_rl_repo",):
    if _p not in sys.path:
        sys.path.append(_p)

import contextlib

import numpy as np
import ml_dtypes

import concourse.bass as bass
import concourse.bacc as bacc
import concourse.mybir as mybir
from concourse.tile import TileContext
from concourse import bass_utils

N, HID, ACT, NH = 256, 128, 5, 4
D, E, HD = 144, 576, 144
NCORES = 8
SH = N // NCORES  # 32
F32 = mybir.dt.float32
BF16 = mybir.dt.bfloat16
SCALE = 1.0 / 12.0
QKM = [(0, 128), (128, 128), (256, 128), (384, 128), (512, 128)]

# blobW bf16 [128, 2432]: WuA(640) Wuh(640) WvA(576) Wvh(576)
W_UA, W_UH, W_VA_, W_VH = 0, 640, 1280, 1280 + E
W_COLS = 1280 + 2 * E
# blobA bf16 [128, 528]: hT(256) aT(256) wenc(16)
A_COLS = 528
# blobF bf16 [128, 60]: Wbig8(48) nvec+bva row0 (12)
F_WB, F_NB = 0, 48
F_COLS = 60
# fside f32 [128, 6]: negx0 negy0 negx1 negy1 jidx0 jidx1
FS_COLS = 6
N_WARM = 10


def _build_fast():
    nc = bacc.Bacc(target_bir_lowering=False)

    def dp(name, shape, dtype, isOutput=False):
        return nc.declare_dram_parameter(name, shape, dtype, isOutput)

    blobA_d = dp("blobA", [128, A_COLS], BF16)
    blobW_d = dp("blobW", [128, W_COLS], BF16)
    blobF_d = dp("blobF", [128, F_COLS], BF16)
    fside_d = dp("fside", [128, FS_COLS], F32)
    crow_d = dp("crow", [3, SH], BF16)          # per-core x, y, idx rows
    out_d = dp("out", [SH, ACT], F32, isOutput=True)

    with TileContext(nc) as tc:
        with contextlib.ExitStack() as ctx:
            wp = ctx.enter_context(tc.tile_pool(name="wp", bufs=1))
            pp = ctx.enter_context(tc.tile_pool(name="pp", bufs=7, space="PSUM"))
            pwp = ctx.enter_context(tc.tile_pool(name="pwp", bufs=1,
                                                 space="PSUM"))

            def wt(shape, tag, dtype=BF16):
                return wp.tile(shape, dtype, tag=tag, name=tag)

            def ps(shape):
                return pp.tile(shape, F32, tag="mm", name="mm")

            # ---------- DMAs: 2 HWDGE queues + SWDGE, critical-first -------
            blobA = wt([128, A_COLS], "blobA")
            blobW = wt([128, W_COLS], "blobW")
            fside = wt([128, FS_COLS], "fside", F32)
            blobF = wt([128, F_COLS], "blobF")
            # sync: aT+wenc, then WuA, then WvA, then blobF
            nc.sync.dma_start(out=blobA[:, 256:A_COLS],
                              in_=blobA_d[:, 256:A_COLS])
            nc.sync.dma_start(out=blobW[:, W_UA:W_UA + 640],
                              in_=blobW_d[:, W_UA:W_UA + 640])
            nc.sync.dma_start(out=blobW[:, W_VA_:W_VA_ + E],
                              in_=blobW_d[:, W_VA_:W_VA_ + E])
            nc.sync.dma_start(out=blobF, in_=blobF_d[:, :])
            # scalar: fside, hT, then Wuh, then Wvh
            nc.scalar.dma_start(out=fside, in_=fside_d[:, :])
            nc.scalar.dma_start(out=blobA[:, 0:256], in_=blobA_d[:, 0:256])
            nc.scalar.dma_start(out=blobW[:, W_UH:W_UH + 640],
                                in_=blobW_d[:, W_UH:W_UH + 640])
            nc.scalar.dma_start(out=blobW[:, W_VH:W_VH + E],
                                in_=blobW_d[:, W_VH:W_VH + E])
            # SWDGE: per-core broadcast rows [128, 32]
            xibc = wt([128, SH], "xibc")
            yibc = wt([128, SH], "yibc")
            idbc = wt([128, SH], "idbc")
            for t, off in ((xibc, 0), (yibc, SH), (idbc, 2 * SH)):
                nc.gpsimd.dma_start(
                    out=t, in_=bass.AP(tensor=crow_d.ap().tensor, offset=off,
                                       ap=[[0, 128], [1, SH]]))

            # ---------- tile views ----------
            hT = blobA[:, 0:256]
            aT = blobA[:, 256:512]
            wenc = blobA[:, 512:528]
            movs = [aT, hT]
            wu_t = [blobW[:, W_UA:W_UA + 640], blobW[:, W_UH:W_UH + 640]]
            wv_t = [blobW[:, W_VA_:W_VA_ + E], blobW[:, W_VH:W_VH + E]]
            wbig_t = [blobF[:, F_WB + 6 * g:F_WB + 6 * g + 6] for g in range(8)]
            nvec_r = blobF[0:1, F_NB:F_NB + 6]
            bva_r = blobF[0:1, F_NB + 6:F_NB + 12]
            negx = [fside[:, 0:1], fside[:, 2:3]]
            negy = [fside[:, 1:2], fside[:, 3:4]]
            jidx = [fside[:, 4:5], fside[:, 5:6]]

            # ---------- PE warmup during the DMA wait ----------
            wup = wt([128, 128], "wup")
            nc.vector.memset(wup, 0.0)
            pw = pwp.tile([128, 128], F32, tag="wu", name="wu")
            for _ in range(N_WARM):
                nc.tensor.matmul(pw, wup, wup, start=True, stop=True)

            # ---------- mcT[km][j, i-local] directly on DVE ----------
            mcT = []
            for km in range(2):
                dx = wt([128, SH], f"dx{km}")
                dy = wt([128, SH], f"dy{km}")
                nc.vector.tensor_scalar(dx, xibc, negx[km], None,
                                        mybir.AluOpType.add)
                nc.vector.tensor_scalar(dy, yibc, negy[km], None,
                                        mybir.AluOpType.add)
                dx2 = wt([128, SH], f"dx2{km}")
                dy2 = wt([128, SH], f"dy2{km}")
                nc.vector.tensor_tensor(dx2, dx, dx, mybir.AluOpType.mult)
                nc.vector.tensor_tensor(dy2, dy, dy, mybir.AluOpType.mult)
                nc.vector.tensor_scalar(dx, dx2, 16.0, None,
                                        mybir.AluOpType.is_le)
                nc.vector.tensor_scalar(dy, dy2, 4.0, None,
                                        mybir.AluOpType.is_le)
                up = wt([128, SH], f"up{km}")
                nc.vector.tensor_scalar(up, idbc, jidx[km], None,
                                        mybir.AluOpType.is_lt)
                pm = wt([128, SH], f"pm{km}")
                nc.vector.tensor_tensor(pm, dx, dy, mybir.AluOpType.mult)
                mk = wt([128, SH], f"mcT{km}")
                nc.vector.tensor_tensor(mk, pm, up, mybir.AluOpType.mult)
                mcT.append(mk)

            # ---------- obsT = W_enc^T hT  [16, 256] ----------
            p0 = ps([16, N])
            nc.tensor.matmul(p0, wenc, hT, start=True, stop=True)
            obsT = wt([16, N], "obsT")
            nc.any.tensor_copy(out=obsT, in_=p0)

            # ---------- uT = Mall^T C^T: 4 mains [128,256] + 4 tails [16,256]
            uTm = []
            uTt = []
            for mi, (ms, ml) in enumerate(QKM):
                p = ps([ml, N])
                for ci in range(2):
                    nc.tensor.matmul(p, wu_t[ci][:, ms:ms + ml], movs[ci],
                                     start=(ci == 0), stop=(ci == 1))
                if mi < 4:
                    t = wt([128, N], f"uTm{mi}")
                    nc.any.tensor_copy(out=t, in_=p)
                    uTm.append(t)
                else:
                    for h in range(4):
                        t = wt([16, N], f"uTt{h}")
                        nc.any.tensor_copy(out=t, in_=p[32 * h:32 * h + 16, :])
                        uTt.append(t)

            # ---------- S_h, S_h^T -> E_h, E_h^T (bf16) ----------
            E_t = [[None, None] for _ in range(NH)]
            ET_t = [[None, None] for _ in range(NH)]
            for h in range(NH):
                for mj in range(2):
                    sl = slice(mj * 128, (mj + 1) * 128)
                    pS = ps([128, N])
                    pST = ps([128, N])
                    nc.tensor.matmul(pS, uTm[h][:, sl], aT,
                                     start=True, stop=False)
                    nc.tensor.matmul(pS, uTt[h][:, sl], obsT,
                                     start=False, stop=True)
                    nc.tensor.matmul(pST, aT[:, sl], uTm[h],
                                     start=True, stop=False)
                    nc.tensor.matmul(pST, obsT[:, sl], uTt[h],
                                     start=False, stop=True)
                    Eh = wt([128, N], f"E{h}_{mj}")
                    ETh = wt([128, N], f"ET{h}_{mj}")
                    nc.scalar.activation(Eh, pS, mybir.ActivationFunctionType.Exp,
                                         scale=SCALE)
                    nc.scalar.activation(ETh, pST,
                                         mybir.ActivationFunctionType.Exp,
                                         scale=SCALE)
                    E_t[h][mj] = Eh
                    ET_t[h][mj] = ETh

            # ---------- v0 = C @ Wvf  [n, E] (bias folded into nvec) ----
            v_t = []
            for nt in range(2):
                vt = wt([128, E], f"v{nt}")
                for ns, nl in ((0, 288), (288, 288)):
                    p = ps([128, nl])
                    for ci in range(2):
                        nc.tensor.matmul(
                            p, movs[ci][:, nt * 128:(nt + 1) * 128],
                            wv_t[ci][:, ns:ns + nl],
                            start=(ci == 0), stop=(ci == 1))
                    nc.any.tensor_copy(out=vt[:, ns:ns + nl], in_=p)
                v_t.append(vt)

            # ---------- n_i^T [1, 32] ----------
            ones_t = wt([128, 1], "ones_t")
            nc.vector.memset(ones_t, 1.0)
            pn = ps([1, SH])
            for c in range(2):
                nc.tensor.matmul(pn, ones_t, mcT[c],
                                 start=(c == 0), stop=(c == 1))
            n_bf = wt([1, SH], "n_bf")
            nc.any.tensor_copy(out=n_bf, in_=pn)

            # ---------- R^T, W^T, ctx ----------
            RT = {}
            for h in range(NH):
                for jm in range(2):
                    p = ps([128, SH])
                    for kc in range(2):
                        nc.tensor.matmul(
                            p, ET_t[h][kc][:, jm * 128:(jm + 1) * 128],
                            mcT[kc], start=(kc == 0), stop=(kc == 1))
                    rtf = wt([128, SH], f"RTf{h}_{jm}", F32)
                    nc.vector.tensor_scalar(rtf, p, 1e-9, None,
                                            mybir.AluOpType.max)
                    with nc.allow_low_precision(reason="attn renorm"):
                        nc.vector.reciprocal(rtf, rtf)
                    rt = wt([128, SH], f"RT{h}_{jm}")
                    nc.vector.tensor_tensor(rt, rtf, mcT[jm],
                                            mybir.AluOpType.mult)
                    RT[(h, jm)] = rt
            WT = {}
            for h in range(NH):
                for km in range(2):
                    p = ps([128, SH])
                    for jc in range(2):
                        nc.tensor.matmul(
                            p, E_t[h][jc][:, km * 128:(km + 1) * 128],
                            RT[(h, jc)], start=(jc == 0), stop=(jc == 1))
                    wtl = wt([128, SH], f"WT{h}_{km}")
                    nc.vector.tensor_tensor(wtl, p, mcT[km],
                                            mybir.AluOpType.mult)
                    WT[(h, km)] = wtl
            ctxT8 = [None] * 8
            for h in range(NH):
                for dm, (ds, dl) in enumerate([(0, 128), (128, 16)]):
                    p = ps([dl, SH])
                    for kc in range(2):
                        nc.tensor.matmul(
                            p, v_t[kc][:, HD * h + ds:HD * h + ds + dl],
                            WT[(h, kc)], start=(kc == 0), stop=(kc == 1))
                    t = wt([dl, SH], f"cT{2 * h + dm}")
                    nc.any.tensor_copy(out=t, in_=p)
                    ctxT8[2 * h + dm] = t

            # ---------- VA = sum_g ctx_g @ Wbig_g + n_i*nvec + 1*bva ----
            ones_r = wt([1, SH], "ones_r")
            nc.vector.memset(ones_r, 1.0)
            pVA = ps([SH, 6])
            for g in range(8):
                dl = 128 if g % 2 == 0 else 16
                nc.tensor.matmul(pVA, ctxT8[g], wbig_t[g][0:dl, :],
                                 start=(g == 0), stop=False)
            nc.tensor.matmul(pVA, n_bf, nvec_r, start=False, stop=False)
            nc.tensor.matmul(pVA, ones_r, bva_r, start=False, stop=True)
            # dueling tail straight off PSUM
            sA = wt([SH, 1], "sA", F32)
            nc.vector.reduce_sum(sA, pVA[:, 1:6], axis=mybir.AxisListType.X)
            vm = wt([SH, 1], "vm", F32)
            nc.vector.scalar_tensor_tensor(out=vm, in0=sA, scalar=-0.2,
                                           in1=pVA[:, 0:1],
                                           op0=mybir.AluOpType.mult,
                                           op1=mybir.AluOpType.add)
            Q_sb = wt([SH, ACT], "Qsb", F32)
            nc.vector.tensor_scalar(Q_sb, pVA[:, 1:6], vm, None,
                                    mybir.AluOpType.add)
            nc.sync.dma_start(out=out_d[:, :], in_=Q_sb)

    nc.compile()
    return nc


_NC_CACHE = {}
BF = ml_dtypes.bfloat16


def _make_in_maps_fast(inputs):
    f32 = np.float32
    g = lambda k: np.asarray(inputs[k], dtype=f32)

    hidden, action = g("hidden_state_n"), g("action_n")
    state = np.asarray(inputs["state_n"]).astype(np.int32)
    W_enc = g("W_enc")

    Wqf = g("Wq") @ g("Wiq")                    # [144, 576]
    Wkf = g("Wk") @ g("Wik")
    # Mall[:, 144h:144h+144] = Qh @ Kh^T  over C-features
    Mall = np.concatenate(
        [Wqf[:, 144 * h:144 * h + 144] @ Wkf[:, 144 * h:144 * h + 144].T
         for h in range(4)], axis=1)            # [144, 576]
    WuA = Mall[16:144]                          # action rows [128, 576]
    Wuh = W_enc @ Mall[0:16]                    # hidden rows [128, 576]

    Wvf = g("Wv") @ g("Wiv")
    WvA = Wvf[16:144]
    Wvh = W_enc @ Wvf[0:16]
    bvf = g("b_enc") @ Wvf[0:16] + g("bv") @ g("Wiv") + g("biv")   # [576]

    Wva6 = np.concatenate([g("W_val").reshape(D, 1),
                           g("W_adv").reshape(D, ACT)], axis=1)    # [144,6]
    WoWO = g("Wo_proj") @ g("W_O")                                 # [576,144]
    Wbig = WoWO @ Wva6                                             # [576,6]
    nvec = bvf @ Wbig + (g("bo_proj") @ g("W_O")) @ Wva6           # [6]
    bva6 = np.concatenate([g("b_val").reshape(1), g("b_adv")])     # [6]

    def padu(w):  # [128, 576] head-blocks [obs16|act128] -> [128, 640]
        mains = [w[:, 144 * h + 16:144 * h + 144] for h in range(4)]
        z = np.zeros((w.shape[0], 16), f32)
        tails = []
        for h in range(4):
            tails += [w[:, 144 * h:144 * h + 16], z]
        return np.concatenate(mains + tails, axis=1)

    blobA = np.concatenate([np.ascontiguousarray(hidden.T),
                            np.ascontiguousarray(action.T), W_enc], axis=1)
    blobW = np.concatenate([padu(WuA), padu(Wuh), WvA, Wvh], axis=1)
    state_f = state.astype(f32)
    fside = np.zeros((128, FS_COLS), f32)
    fside[:, 0] = -state_f[0:128, 0]
    fside[:, 1] = -state_f[0:128, 1]
    fside[:, 2] = -state_f[128:256, 0]
    fside[:, 3] = -state_f[128:256, 1]
    fside[:, 4] = np.arange(128, dtype=f32)
    fside[:, 5] = np.arange(128, 256, dtype=f32)

    wbig8 = np.zeros((128, 48), f32)
    for h in range(4):
        wbig8[:, 12 * h:12 * h + 6] = Wbig[144 * h:144 * h + 128]
        wbig8[0:16, 12 * h + 6:12 * h + 12] = Wbig[144 * h + 128:144 * (h + 1)]
    nbrow = np.zeros((128, 12), f32)
    nbrow[0, 0:6] = nvec
    nbrow[0, 6:12] = bva6

    shared = {
        "blobA": blobA.astype(BF),
        "blobW": blobW.astype(BF),
        "fside": fside,
    }
    in_maps = []
    for c in range(NCORES):
        bF = np.concatenate([wbig8, nbrow], axis=1)
        crow = np.stack([state_f[c * SH:(c + 1) * SH, 0],
                         state_f[c * SH:(c + 1) * SH, 1],
                         np.arange(c * SH, (c + 1) * SH, dtype=f32)])
        m = dict(shared)
        m["blobF"] = np.ascontiguousarray(bF).astype(BF)
        m["crow"] = np.ascontiguousarray(crow).astype(BF)
        in_maps.append(m)
    return in_maps


def _zero_qk_bias(inputs):
    return all(not np.any(np.asarray(inputs[k]))
               for k in ("bq", "bk", "biq", "bik"))


def kernel(**inputs):
    if not _zero_qk_bias(inputs):
        return _kernel_general(inputs)
    if "fast" not in _NC_CACHE:
        _NC_CACHE["fast"] = _build_fast()
    nc = _NC_CACHE["fast"]
    in_maps = _make_in_maps_fast(inputs)
    res = bass_utils.run_bass_kernel_spmd(nc, in_maps,
                                          core_ids=list(range(NCORES)))
    return np.concatenate([res.results[c]["out"] for c in range(NCORES)],
                          axis=0)


# ======================= general path (nonzero q/k biases) ==================
EPg = 640
QKMg = [(0, 128), (128, 128), (256, 128), (384, 128), (512, 64), (576, 64)]
GW_QA, GW_QH, GW_KA, GW_KH, GW_VA, GW_VH = (0, EPg, 2 * EPg, 3 * EPg,
                                            4 * EPg, 4 * EPg + E)
GW_COLS = 4 * EPg + 2 * E
GF_SEL, GF_WB = 0, 64
GF_COLS = GF_WB + 48
GFS_NEG, GFS_BQ, GFS_BK = 0, 4, 10
GFS_COLS = 16


def _build_general():
    nc = bacc.Bacc(target_bir_lowering=False)

    def dp(name, shape, dtype, isOutput=False):
        return nc.declare_dram_parameter(name, shape, dtype, isOutput)

    blobA_d = dp("blobA", [128, 512], BF16)
    blobW_d = dp("blobW", [128, GW_COLS], BF16)
    blobF_d = dp("blobF", [128, GF_COLS], BF16)
    fside_d = dp("fside", [128, GFS_COLS], F32)
    srows_d = dp("srows", [2, N], BF16)
    nb2_d = dp("nb2", [1, 12], F32)
    out_d = dp("out", [SH, ACT], F32, isOutput=True)

    with TileContext(nc) as tc:
        with contextlib.ExitStack() as ctx:
            wp = ctx.enter_context(tc.tile_pool(name="wp", bufs=1))
            pp = ctx.enter_context(tc.tile_pool(name="pp", bufs=7, space="PSUM"))

            def wt(shape, tag, dtype=BF16):
                return wp.tile(shape, dtype, tag=tag, name=tag)

            def ps(shape):
                return pp.tile(shape, F32, tag="mm", name="mm")

            dma = nc.sync.dma_start
            sdma = nc.scalar.dma_start
            gdma = nc.gpsimd.dma_start

            blobA = wt([128, 512], "blobA")
            dma(out=blobA, in_=blobA_d[:, :])
            blobW = wt([128, GW_COLS], "blobW")
            dma(out=blobW[:, 0:2 * EPg], in_=blobW_d[:, 0:2 * EPg])
            fside = wt([128, GFS_COLS], "fside", F32)
            sdma(out=fside, in_=fside_d[:, :])
            sdma(out=blobW[:, 2 * EPg:4 * EPg], in_=blobW_d[:, 2 * EPg:4 * EPg])
            dma(out=blobW[:, 4 * EPg:GW_COLS], in_=blobW_d[:, 4 * EPg:GW_COLS])
            blobF = wt([128, GF_COLS], "blobF")
            sdma(out=blobF, in_=blobF_d[:, :])
            xjf = wt([128, N], "xjf")
            gdma(out=xjf, in_=bass.AP(tensor=srows_d.ap().tensor, offset=0,
                                      ap=[[0, 128], [1, N]]))
            yjf = wt([128, N], "yjf")
            gdma(out=yjf, in_=bass.AP(tensor=srows_d.ap().tensor, offset=N,
                                      ap=[[0, 128], [1, N]]))
            nb_bc = wt([SH, 12], "nbbc", F32)
            gdma(out=nb_bc, in_=bass.AP(tensor=nb2_d.ap().tensor, offset=0,
                                        ap=[[0, SH], [1, 12]]))

            hT = blobA[:, 0:256]
            aT = blobA[:, 256:512]
            movs = [aT, hT]
            wq_t = [blobW[:, GW_QA:GW_QA + EPg], blobW[:, GW_QH:GW_QH + EPg]]
            wk_t = [blobW[:, GW_KA:GW_KA + EPg], blobW[:, GW_KH:GW_KH + EPg]]
            wv_t = [blobW[:, GW_VA:GW_VA + E], blobW[:, GW_VH:GW_VH + E]]
            sel_t = [blobF[:, GF_SEL:GF_SEL + SH],
                     blobF[:, GF_SEL + SH:GF_SEL + 2 * SH]]
            wbig_t = [blobF[:, GF_WB + 6 * g:GF_WB + 6 * g + 6]
                      for g in range(8)]
            negx = [fside[:, 0:1], fside[:, 2:3]]
            negy = [fside[:, 1:2], fside[:, 3:4]]
            bq_t = [fside[:, GFS_BQ + h:GFS_BQ + h + 1] for h in range(4)] + \
                   [fside[0:64, GFS_BQ + 4:GFS_BQ + 5],
                    fside[0:64, GFS_BQ + 5:GFS_BQ + 6]]
            bk_t = [fside[:, GFS_BK + h:GFS_BK + h + 1] for h in range(4)] + \
                   [fside[0:64, GFS_BK + 4:GFS_BK + 5],
                    fside[0:64, GFS_BK + 5:GFS_BK + 6]]

            mask_t = []
            for c in range(2):
                bx = wt([128, N], f"bx{c}")
                by = wt([128, N], f"by{c}")
                nc.vector.tensor_scalar(bx, xjf, negx[c], None,
                                        mybir.AluOpType.add)
                nc.vector.tensor_scalar(by, yjf, negy[c], None,
                                        mybir.AluOpType.add)
                bx2 = wt([128, N], f"bx2{c}")
                by2 = wt([128, N], f"by2{c}")
                nc.vector.tensor_tensor(bx2, bx, bx, mybir.AluOpType.mult)
                nc.vector.tensor_tensor(by2, by, by, mybir.AluOpType.mult)
                nc.vector.tensor_scalar(bx, bx2, 16.0, None,
                                        mybir.AluOpType.is_le)
                nc.vector.tensor_scalar(by, by2, 4.0, None,
                                        mybir.AluOpType.is_le)
                prox = wt([128, N], f"prox{c}")
                nc.vector.tensor_tensor(prox, bx, by, mybir.AluOpType.mult)
                mk = wt([128, N], f"mask{c}")
                nc.gpsimd.affine_select(out=mk, in_=prox, pattern=[[1, N]],
                                        compare_op=mybir.AluOpType.is_gt,
                                        fill=0.0, base=-c * 128,
                                        channel_multiplier=-1)
                mask_t.append(mk)

            def proj(w_t, b_t, tag):
                outs = []
                for mi, (ms, ml) in enumerate(QKMg):
                    p = ps([ml, N])
                    for ci in range(2):
                        nc.tensor.matmul(p, w_t[ci][:, ms:ms + ml], movs[ci],
                                         start=(ci == 0), stop=(ci == 1))
                    t = wt([ml, N], f"{tag}{mi}")
                    nc.vector.tensor_scalar(t, p, b_t[mi], None,
                                            mybir.AluOpType.add)
                    outs.append(t)
                return outs

            qT = proj(wq_t, bq_t, "qT")
            kT = proj(wk_t, bk_t, "kT")

            E_t = [[None, None] for _ in range(NH)]
            ET_t = [[None, None] for _ in range(NH)]
            for h in range(NH):
                hs = [(h, 0, 128), (4 + h // 2, 32 * (h % 2), 32)]
                for mj in range(2):
                    pS = ps([128, N])
                    pST = ps([128, N])
                    for ci, (ti, rs, rl) in enumerate(hs):
                        st_, sp = (ci == 0), (ci == 1)
                        nc.tensor.matmul(
                            pS, qT[ti][rs:rs + rl, mj * 128:(mj + 1) * 128],
                            kT[ti][rs:rs + rl, :], start=st_, stop=sp)
                        nc.tensor.matmul(
                            pST, kT[ti][rs:rs + rl, mj * 128:(mj + 1) * 128],
                            qT[ti][rs:rs + rl, :], start=st_, stop=sp)
                    Eh = wt([128, N], f"E{h}_{mj}")
                    ETh = wt([128, N], f"ET{h}_{mj}")
                    nc.scalar.activation(Eh, pS,
                                         mybir.ActivationFunctionType.Exp,
                                         scale=SCALE)
                    nc.scalar.activation(ETh, pST,
                                         mybir.ActivationFunctionType.Exp,
                                         scale=SCALE)
                    E_t[h][mj] = Eh
                    ET_t[h][mj] = ETh

            v_t = []
            for nt in range(2):
                vt = wt([128, E], f"v{nt}")
                for ns, nl in ((0, 288), (288, 288)):
                    p = ps([128, nl])
                    for ci in range(2):
                        nc.tensor.matmul(
                            p, movs[ci][:, nt * 128:(nt + 1) * 128],
                            wv_t[ci][:, ns:ns + nl],
                            start=(ci == 0), stop=(ci == 1))
                    nc.any.tensor_copy(out=vt[:, ns:ns + nl], in_=p)
                v_t.append(vt)

            mcT = []
            for km in range(2):
                p = ps([128, SH])
                for c in range(2):
                    nc.tensor.matmul(
                        p, mask_t[c][:, km * 128:(km + 1) * 128],
                        sel_t[c], start=(c == 0), stop=(c == 1))
                t = wt([128, SH], f"mcT{km}")
                nc.any.tensor_copy(out=t, in_=p)
                mcT.append(t)
            ones_t = wt([128, 1], "ones_t")
            nc.vector.memset(ones_t, 1.0)
            pn = ps([SH, 1])
            for c in range(2):
                nc.tensor.matmul(pn, mcT[c], ones_t,
                                 start=(c == 0), stop=(c == 1))
            n_i = wt([SH, 1], "n_i", F32)
            nc.any.tensor_copy(out=n_i, in_=pn)

            RT = {}
            for h in range(NH):
                for jm in range(2):
                    p = ps([128, SH])
                    for kc in range(2):
                        nc.tensor.matmul(
                            p, ET_t[h][kc][:, jm * 128:(jm + 1) * 128],
                            mcT[kc], start=(kc == 0), stop=(kc == 1))
                    rtf = wt([128, SH], f"RTf{h}_{jm}", F32)
                    nc.vector.tensor_scalar(rtf, p, 1e-9, None,
                                            mybir.AluOpType.max)
                    with nc.allow_low_precision(reason="attn renorm"):
                        nc.vector.reciprocal(rtf, rtf)
                    rt = wt([128, SH], f"RT{h}_{jm}")
                    nc.vector.tensor_tensor(rt, rtf, mcT[jm],
                                            mybir.AluOpType.mult)
                    RT[(h, jm)] = rt
            WT = {}
            for h in range(NH):
                for km in range(2):
                    p = ps([128, SH])
                    for jc in range(2):
                        nc.tensor.matmul(
                            p, E_t[h][jc][:, km * 128:(km + 1) * 128],
                            RT[(h, jc)], start=(jc == 0), stop=(jc == 1))
                    wtl = wt([128, SH], f"WT{h}_{km}")
                    nc.vector.tensor_tensor(wtl, p, mcT[km],
                                            mybir.AluOpType.mult)
                    WT[(h, km)] = wtl
            ctxT8 = [None] * 8
            for h in range(NH):
                for dm, (ds, dl) in enumerate([(0, 128), (128, 16)]):
                    p = ps([dl, SH])
                    for kc in range(2):
                        nc.tensor.matmul(
                            p, v_t[kc][:, HD * h + ds:HD * h + ds + dl],
                            WT[(h, kc)], start=(kc == 0), stop=(kc == 1))
                    t = wt([dl, SH], f"cT{2 * h + dm}")
                    nc.any.tensor_copy(out=t, in_=p)
                    ctxT8[2 * h + dm] = t

            pVA = ps([SH, 6])
            for g in range(8):
                dl = 128 if g % 2 == 0 else 16
                nc.tensor.matmul(pVA, ctxT8[g], wbig_t[g][0:dl, :],
                                 start=(g == 0), stop=(g == 7))
            VAt = wt([SH, 6], "VAt", F32)
            nc.vector.scalar_tensor_tensor(
                out=VAt, in0=nb_bc[:, 0:6], scalar=n_i, in1=pVA,
                op0=mybir.AluOpType.mult, op1=mybir.AluOpType.add)
            VA = wt([SH, 6], "VA", F32)
            nc.vector.tensor_tensor(VA, VAt, nb_bc[:, 6:12],
                                    mybir.AluOpType.add)
            sA = wt([SH, 1], "sA", F32)
            nc.vector.reduce_sum(sA, VA[:, 1:6], axis=mybir.AxisListType.X)
            vm = wt([SH, 1], "vm", F32)
            nc.vector.scalar_tensor_tensor(out=vm, in0=sA, scalar=-0.2,
                                           in1=VA[:, 0:1],
                                           op0=mybir.AluOpType.mult,
                                           op1=mybir.AluOpType.add)
            Q_sb = wt([SH, ACT], "Qsb", F32)
            nc.vector.tensor_scalar(Q_sb, VA[:, 1:6], vm, None,
                                    mybir.AluOpType.add)
            nc.gpsimd.dma_start(out=out_d[:, :], in_=Q_sb)

    nc.compile()
    return nc


def _make_in_maps_general(inputs):
    f32 = np.float32
    g = lambda k: np.asarray(inputs[k], dtype=f32)

    hidden, action = g("hidden_state_n"), g("action_n")
    state = np.asarray(inputs["state_n"]).astype(np.int32)
    W_enc, b_enc = g("W_enc"), g("b_enc")
    Wiq, Wik, Wiv = g("Wiq"), g("Wik"), g("Wiv")

    def fuse(Wo_, bo_, Wi_, bi_):
        Wf = Wo_ @ Wi_
        WA = Wf[16:144]
        Wh = W_enc @ Wf[0:16]
        bf = b_enc @ Wf[0:16] + bo_ @ Wi_ + bi_
        return WA, Wh, bf

    WqA, Wqh, bqf = fuse(g("Wq"), g("bq"), Wiq, g("biq"))
    WkA, Wkh, bkf = fuse(g("Wk"), g("bk"), Wik, g("bik"))
    WvA, Wvh, bvf = fuse(g("Wv"), g("bv"), Wiv, g("biv"))

    Wva6 = np.concatenate([g("W_val").reshape(D, 1),
                           g("W_adv").reshape(D, ACT)], axis=1)
    WoWO = g("Wo_proj") @ g("W_O")
    Wbig = WoWO @ Wva6
    nvec = bvf @ Wbig + (g("bo_proj") @ g("W_O")) @ Wva6
    bva6 = np.concatenate([g("b_val").reshape(1), g("b_adv")])

    def padqk(w):
        mains = [w[:, 144 * h:144 * h + 128] for h in range(4)]
        z = np.zeros((w.shape[0], 16), f32)
        tails = [np.concatenate([w[:, 144 * h + 128:144 * h + 144], z,
                                 w[:, 144 * (h + 1) + 128:144 * (h + 1) + 144],
                                 z], axis=1) for h in (0, 2)]
        return np.concatenate(mains + tails, axis=1)

    def bias_cols(b):
        cols = np.zeros((128, 6), f32)
        for h in range(4):
            cols[:, h] = b[144 * h:144 * h + 128]
        for t, h in enumerate((0, 2)):
            cols[0:16, 4 + t] = b[144 * h + 128:144 * h + 144]
            cols[32:48, 4 + t] = b[144 * (h + 1) + 128:144 * (h + 1) + 144]
        return cols

    blobA = np.concatenate([np.ascontiguousarray(hidden.T),
                            np.ascontiguousarray(action.T)], axis=1)
    blobW = np.concatenate([padqk(WqA), padqk(Wqh), padqk(WkA), padqk(Wkh),
                            WvA, Wvh], axis=1)
    state_f = state.astype(f32)
    fside = np.zeros((128, GFS_COLS), f32)
    fside[:, 0] = -state_f[0:128, 0]
    fside[:, 1] = -state_f[0:128, 1]
    fside[:, 2] = -state_f[128:256, 0]
    fside[:, 3] = -state_f[128:256, 1]
    fside[:, GFS_BQ:GFS_BQ + 6] = bias_cols(bqf)
    fside[:, GFS_BK:GFS_BK + 6] = bias_cols(bkf)
    srows = np.ascontiguousarray(state_f.T)
    nb2 = np.concatenate([nvec, bva6]).reshape(1, 12).astype(f32)

    wbig8 = np.zeros((128, 48), f32)
    for h in range(4):
        wbig8[:, 12 * h:12 * h + 6] = Wbig[144 * h:144 * h + 128]
        wbig8[0:16, 12 * h + 6:12 * h + 12] = Wbig[144 * h + 128:144 * (h + 1)]

    eye = np.eye(N, dtype=f32)
    shared = {
        "blobA": blobA.astype(BF),
        "blobW": blobW.astype(BF),
        "fside": fside,
        "srows": srows.astype(BF),
        "nb2": nb2,
    }
    in_maps = []
    for c in range(NCORES):
        sel = eye[:, c * SH:(c + 1) * SH]
        selpack = np.concatenate([sel[0:128], sel[128:256]], axis=1)
        bF = np.concatenate([selpack, wbig8], axis=1)
        m = dict(shared)
        m["blobF"] = np.ascontiguousarray(bF).astype(BF)
        in_maps.append(m)
    return in_maps


def _kernel_general(inputs):
    if "gen" not in _NC_CACHE:
        _NC_CACHE["gen"] = _build_general()
    nc = _NC_CACHE["gen"]
    in_maps = _make_in_maps_general(inputs)
    res = bass_utils.run_bass_kernel_spmd(nc, in_maps,
                                          core_ids=list(range(NCORES)))
    return np.concatenate([res.results[c]["out"] for c in range(NCORES)],
                          axis=0)


# revision 17
# speedup vs baseline: 3.4349x; 1.1170x over previous
"""AttentionCritic Trainium2 kernel — 8-core SPMD, no collectives.

Fast path (zero q/k biases, which setup_inputs produces):
  S_h = q_h k_h^T / 12 = C Mh C^T / 12,  Mh = (Wq Wiq)_h (Wk Wik)_h^T  (host)
  C = [obs, action], obs folded via W_enc into the u-projection weights:
  uT = Mall^T C^T computed as  WuA^T aT + Wuh^T hT   (no k projection at all)
  E_h = exp(S_h/12);  D[i,h,j] = sum_k E_h[j,k] m[i,k];  R = m/max(D,1e-9)
  W[i,h,k] = m[i,k] * sum_j R[i,h,j] E_h[j,k];  ctx0[i,h] = sum_k W v0_h[k]
  VA = sum_h ctx0_h @ Wbig_h + n_i*nvec + bva   (rank-1 PE updates)
  Q = V + A - mean(A)

mask computed directly in transposed [j, i-local] form on DVE from per-core
broadcast rows (x, y, global index) and per-partition j coords; n_i via PE.
All matmul inputs bf16 (fp32 PSUM); DMAs split over 4 HWDGE queues.
General path (nonzero q/k biases): separate q/k projections, same phase B.
"""

import sys

for _p in ("/opt/trn_rl_repo",):
    if _p not in sys.path:
        sys.path.append(_p)

import contextlib

import numpy as np
import ml_dtypes

import concourse.bass as bass
import concourse.bacc as bacc
import concourse.mybir as mybir
from concourse.tile import TileContext
from concourse import bass_utils

N, HID, ACT, NH = 256, 128, 5, 4
D, E, HD = 144, 576, 144
NCORES = 8
SH = N // NCORES  # 32
F32 = mybir.dt.float32
BF16 = mybir.dt.bfloat16
SCALE = 1.0 / 12.0
QKM = [(0, 128), (128, 128), (256, 128), (384, 128), (512, 128)]

# blobW bf16 [128, 2432]: WuA(640) Wuh(640) WvA(576) Wvh(576)
W_UA, W_UH, W_VA_, W_VH = 0, 640, 1280, 1280 + E
W_COLS = 1280 + 2 * E
# blobA bf16 [128, 656]: hT(256) aT(256) wenc(16) id128(128)
A_COLS = 656
# blobF bf16 [128, 60]: Wbig8(48) nvec+bva row0 (12)
F_WB, F_NB = 0, 48
F_COLS = 60
# fside f32 [128, 6]: negx0 negy0 negx1 negy1 jidx0 jidx1
FS_COLS = 6
N_WARM = 14


def _build_fast():
    nc = bacc.Bacc(target_bir_lowering=False)

    def dp(name, shape, dtype, isOutput=False):
        return nc.declare_dram_parameter(name, shape, dtype, isOutput)

    blobA_d = dp("blobA", [128, A_COLS], BF16)
    blobW_d = dp("blobW", [128, W_COLS], BF16)
    blobF_d = dp("blobF", [128, F_COLS], BF16)
    fside_d = dp("fside", [128, FS_COLS], F32)
    crow_d = dp("crow", [3, SH], BF16)          # per-core x, y, idx rows
    out_d = dp("out", [SH, ACT], F32, isOutput=True)

    with TileContext(nc) as tc:
        with contextlib.ExitStack() as ctx:
            wp = ctx.enter_context(tc.tile_pool(name="wp", bufs=1))
            pp = ctx.enter_context(tc.tile_pool(name="pp", bufs=6, space="PSUM"))
            ptt = ctx.enter_context(tc.tile_pool(name="ptt", bufs=2,
                                                 space="PSUM"))

            def wt(shape, tag, dtype=BF16):
                return wp.tile(shape, dtype, tag=tag, name=tag)

            def ps(shape):
                return pp.tile(shape, F32, tag="mm", name="mm")

            # ---------- DMAs: 2 HWDGE queues + SWDGE, critical-first -------
            blobA = wt([128, A_COLS], "blobA")
            blobW = wt([128, W_COLS], "blobW")
            fside = wt([128, FS_COLS], "fside", F32)
            blobF = wt([128, F_COLS], "blobF")
            # sync: aT+wenc, WuA (first tile split out), WvA, id128, blobF
            nc.sync.dma_start(out=blobA[:, 256:528], in_=blobA_d[:, 256:528])
            nc.sync.dma_start(out=blobW[:, W_UA:W_UA + 128],
                              in_=blobW_d[:, W_UA:W_UA + 128])
            nc.sync.dma_start(out=blobW[:, W_UA + 128:W_UA + 640],
                              in_=blobW_d[:, W_UA + 128:W_UA + 640])
            nc.sync.dma_start(out=blobW[:, W_VA_:W_VA_ + E],
                              in_=blobW_d[:, W_VA_:W_VA_ + E])
            nc.sync.dma_start(out=blobA[:, 528:A_COLS],
                              in_=blobA_d[:, 528:A_COLS])
            nc.sync.dma_start(out=blobF, in_=blobF_d[:, :])
            # scalar: fside, hT, Wuh (first tile split out), Wvh
            nc.scalar.dma_start(out=fside, in_=fside_d[:, :])
            nc.scalar.dma_start(out=blobA[:, 0:256], in_=blobA_d[:, 0:256])
            nc.scalar.dma_start(out=blobW[:, W_UH:W_UH + 128],
                                in_=blobW_d[:, W_UH:W_UH + 128])
            nc.scalar.dma_start(out=blobW[:, W_UH + 128:W_UH + 640],
                                in_=blobW_d[:, W_UH + 128:W_UH + 640])
            nc.scalar.dma_start(out=blobW[:, W_VH:W_VH + E],
                                in_=blobW_d[:, W_VH:W_VH + E])
            # SWDGE: per-core broadcast rows [128, 32]
            xibc = wt([128, SH], "xibc")
            yibc = wt([128, SH], "yibc")
            idbc = wt([128, SH], "idbc")
            for t, off in ((xibc, 0), (yibc, SH), (idbc, 2 * SH)):
                nc.gpsimd.dma_start(
                    out=t, in_=bass.AP(tensor=crow_d.ap().tensor, offset=off,
                                       ap=[[0, 128], [1, SH]]))

            # ---------- tile views ----------
            hT = blobA[:, 0:256]
            aT = blobA[:, 256:512]
            wenc = blobA[:, 512:528]
            id128 = blobA[:, 528:656]
            movs = [aT, hT]
            wu_t = [blobW[:, W_UA:W_UA + 640], blobW[:, W_UH:W_UH + 640]]
            wv_t = [blobW[:, W_VA_:W_VA_ + E], blobW[:, W_VH:W_VH + E]]
            wbig_t = [blobF[:, F_WB + 6 * g:F_WB + 6 * g + 6] for g in range(8)]
            nvec_r = blobF[0:1, F_NB:F_NB + 6]
            bva_r = blobF[0:1, F_NB + 6:F_NB + 12]
            negx = [fside[:, 0:1], fside[:, 2:3]]
            negy = [fside[:, 1:2], fside[:, 3:4]]
            jidx = [fside[:, 4:5], fside[:, 5:6]]

            # ---------- PE warmup during the DMA wait ----------
            wup = wt([128, 128], "wup")
            nc.vector.memset(wup, 0.0)
            for _ in range(N_WARM):
                pw = ps([128, 128])
                nc.tensor.matmul(pw, wup, wup, start=True, stop=True)

            # ---------- mcT[km][j, i-local] directly on DVE ----------
            mcT = []
            for km in range(2):
                dx = wt([128, SH], f"dx{km}")
                dy = wt([128, SH], f"dy{km}")
                nc.vector.tensor_scalar(dx, xibc, negx[km], None,
                                        mybir.AluOpType.add)
                nc.vector.tensor_scalar(dy, yibc, negy[km], None,
                                        mybir.AluOpType.add)
                dx2 = wt([128, SH], f"dx2{km}")
                dy2 = wt([128, SH], f"dy2{km}")
                nc.vector.tensor_tensor(dx2, dx, dx, mybir.AluOpType.mult)
                nc.vector.tensor_tensor(dy2, dy, dy, mybir.AluOpType.mult)
                nc.vector.tensor_scalar(dx, dx2, 16.0, None,
                                        mybir.AluOpType.is_le)
                nc.vector.tensor_scalar(dy, dy2, 4.0, None,
                                        mybir.AluOpType.is_le)
                up = wt([128, SH], f"up{km}")
                nc.vector.tensor_scalar(up, idbc, jidx[km], None,
                                        mybir.AluOpType.is_lt)
                pm = wt([128, SH], f"pm{km}")
                nc.vector.tensor_tensor(pm, dx, dy, mybir.AluOpType.mult)
                mk = wt([128, SH], f"mcT{km}")
                nc.vector.tensor_tensor(mk, pm, up, mybir.AluOpType.mult)
                mcT.append(mk)

            # ---------- obsT = W_enc^T hT  [16, 256] ----------
            p0 = ps([16, N])
            nc.tensor.matmul(p0, wenc, hT, start=True, stop=True)
            obsT = wt([16, N], "obsT")
            nc.any.tensor_copy(out=obsT, in_=p0)

            # ---------- uT = Mall^T C^T: 4 mains [128,256] + 4 tails [16,256]
            uTm = []
            uTt = []
            for mi, (ms, ml) in enumerate(QKM):
                p = ps([ml, N])
                for ci in range(2):
                    nc.tensor.matmul(p, wu_t[ci][:, ms:ms + ml], movs[ci],
                                     start=(ci == 0), stop=(ci == 1))
                if mi < 4:
                    t = wt([128, N], f"uTm{mi}")
                    nc.any.tensor_copy(out=t, in_=p)
                    uTm.append(t)
                else:
                    for h in range(4):
                        t = wt([16, N], f"uTt{h}")
                        nc.any.tensor_copy(out=t, in_=p[32 * h:32 * h + 16, :])
                        uTt.append(t)

            # ---------- S_h -> E_h (bf16); E_h^T via PE transpose ----------
            E_t = [[None, None] for _ in range(NH)]
            ET_t = [[None, None] for _ in range(NH)]
            for h in range(NH):
                for mj in range(2):
                    sl = slice(mj * 128, (mj + 1) * 128)
                    pS = ps([128, N])
                    nc.tensor.matmul(pS, uTm[h][:, sl], aT,
                                     start=True, stop=False)
                    nc.tensor.matmul(pS, uTt[h][:, sl], obsT,
                                     start=False, stop=True)
                    Eh = wt([128, N], f"E{h}_{mj}")
                    nc.scalar.activation(Eh, pS, mybir.ActivationFunctionType.Exp,
                                         scale=SCALE)
                    E_t[h][mj] = Eh
            for h in range(NH):
                for kb in range(2):
                    ETh = wt([128, N], f"ET{h}_{kb}")
                    for mj in range(2):
                        pt = ptt.tile([128, 128], BF16, tag="tt", name="tt")
                        nc.tensor.transpose(
                            pt, E_t[h][mj][:, kb * 128:(kb + 1) * 128], id128)
                        nc.any.tensor_copy(
                            out=ETh[:, mj * 128:(mj + 1) * 128], in_=pt)
                    ET_t[h][kb] = ETh

            # ---------- v0 = C @ Wvf  [n, E] (bias folded into nvec) ----
            v_t = []
            for nt in range(2):
                vt = wt([128, E], f"v{nt}")
                for ns, nl in ((0, 288), (288, 288)):
                    p = ps([128, nl])
                    for ci in range(2):
                        nc.tensor.matmul(
                            p, movs[ci][:, nt * 128:(nt + 1) * 128],
                            wv_t[ci][:, ns:ns + nl],
                            start=(ci == 0), stop=(ci == 1))
                    nc.any.tensor_copy(out=vt[:, ns:ns + nl], in_=p)
                v_t.append(vt)

            # ---------- n_i^T [1, 32] ----------
            ones_t = wt([128, 1], "ones_t")
            nc.vector.memset(ones_t, 1.0)
            pn = ps([1, SH])
            for c in range(2):
                nc.tensor.matmul(pn, ones_t, mcT[c],
                                 start=(c == 0), stop=(c == 1))
            n_bf = wt([1, SH], "n_bf")
            nc.any.tensor_copy(out=n_bf, in_=pn)

            # ---------- R^T, W^T, ctx ----------
            RT = {}
            for h in range(NH):
                for jm in range(2):
                    p = ps([128, SH])
                    for kc in range(2):
                        nc.tensor.matmul(
                            p, ET_t[h][kc][:, jm * 128:(jm + 1) * 128],
                            mcT[kc], start=(kc == 0), stop=(kc == 1))
                    rtf = wt([128, SH], f"RTf{h}_{jm}", F32)
                    nc.vector.tensor_scalar(rtf, p, 1e-9, None,
                                            mybir.AluOpType.max)
                    with nc.allow_low_precision(reason="attn renorm"):
                        nc.vector.reciprocal(rtf, rtf)
                    rt = wt([128, SH], f"RT{h}_{jm}")
                    nc.vector.tensor_tensor(rt, rtf, mcT[jm],
                                            mybir.AluOpType.mult)
                    RT[(h, jm)] = rt
            WT = {}
            for h in range(NH):
                for km in range(2):
                    p = ps([128, SH])
                    for jc in range(2):
                        nc.tensor.matmul(
                            p, E_t[h][jc][:, km * 128:(km + 1) * 128],
                            RT[(h, jc)], start=(jc == 0), stop=(jc == 1))
                    wtl = wt([128, SH], f"WT{h}_{km}")
                    nc.vector.tensor_tensor(wtl, p, mcT[km],
                                            mybir.AluOpType.mult)
                    WT[(h, km)] = wtl
            ctxT8 = [None] * 8
            for h in range(NH):
                for dm, (ds, dl) in enumerate([(0, 128), (128, 16)]):
                    p = ps([dl, SH])
                    for kc in range(2):
                        nc.tensor.matmul(
                            p, v_t[kc][:, HD * h + ds:HD * h + ds + dl],
                            WT[(h, kc)], start=(kc == 0), stop=(kc == 1))
                    t = wt([dl, SH], f"cT{2 * h + dm}")
                    nc.any.tensor_copy(out=t, in_=p)
                    ctxT8[2 * h + dm] = t

            # ---------- VA = sum_g ctx_g @ Wbig_g + n_i*nvec + 1*bva ----
            ones_r = wt([1, SH], "ones_r")
            nc.vector.memset(ones_r, 1.0)
            pVA = ps([SH, 6])
            for g in range(8):
                dl = 128 if g % 2 == 0 else 16
                nc.tensor.matmul(pVA, ctxT8[g], wbig_t[g][0:dl, :],
                                 start=(g == 0), stop=False)
            nc.tensor.matmul(pVA, n_bf, nvec_r, start=False, stop=False)
            nc.tensor.matmul(pVA, ones_r, bva_r, start=False, stop=True)
            # dueling tail straight off PSUM
            sA = wt([SH, 1], "sA", F32)
            nc.vector.reduce_sum(sA, pVA[:, 1:6], axis=mybir.AxisListType.X)
            vm = wt([SH, 1], "vm", F32)
            nc.vector.scalar_tensor_tensor(out=vm, in0=sA, scalar=-0.2,
                                           in1=pVA[:, 0:1],
                                           op0=mybir.AluOpType.mult,
                                           op1=mybir.AluOpType.add)
            Q_sb = wt([SH, ACT], "Qsb", F32)
            nc.vector.tensor_scalar(Q_sb, pVA[:, 1:6], vm, None,
                                    mybir.AluOpType.add)
            nc.sync.dma_start(out=out_d[:, :], in_=Q_sb)

    nc.compile()
    return nc


_NC_CACHE = {}
BF = ml_dtypes.bfloat16


def _make_in_maps_fast(inputs):
    f32 = np.float32
    g = lambda k: np.asarray(inputs[k], dtype=f32)

    hidden, action = g("hidden_state_n"), g("action_n")
    state = np.asarray(inputs["state_n"]).astype(np.int32)
    W_enc = g("W_enc")

    Wqf = g("Wq") @ g("Wiq")                    # [144, 576]
    Wkf = g("Wk") @ g("Wik")
    # Mall[:, 144h:144h+144] = Qh @ Kh^T  over C-features
    Mall = np.concatenate(
        [Wqf[:, 144 * h:144 * h + 144] @ Wkf[:, 144 * h:144 * h + 144].T
         for h in range(4)], axis=1)            # [144, 576]
    WuA = Mall[16:144]                          # action rows [128, 576]
    Wuh = W_enc @ Mall[0:16]                    # hidden rows [128, 576]

    Wvf = g("Wv") @ g("Wiv")
    WvA = Wvf[16:144]
    Wvh = W_enc @ Wvf[0:16]
    bvf = g("b_enc") @ Wvf[0:16] + g("bv") @ g("Wiv") + g("biv")   # [576]

    Wva6 = np.concatenate([g("W_val").reshape(D, 1),
                           g("W_adv").reshape(D, ACT)], axis=1)    # [144,6]
    WoWO = g("Wo_proj") @ g("W_O")                                 # [576,144]
    Wbig = WoWO @ Wva6                                             # [576,6]
    nvec = bvf @ Wbig + (g("bo_proj") @ g("W_O")) @ Wva6           # [6]
    bva6 = np.concatenate([g("b_val").reshape(1), g("b_adv")])     # [6]

    def padu(w):  # [128, 576] head-blocks [obs16|act128] -> [128, 640]
        mains = [w[:, 144 * h + 16:144 * h + 144] for h in range(4)]
        z = np.zeros((w.shape[0], 16), f32)
        tails = []
        for h in range(4):
            tails += [w[:, 144 * h:144 * h + 16], z]
        return np.concatenate(mains + tails, axis=1)

    blobA = np.concatenate([np.ascontiguousarray(hidden.T),
                            np.ascontiguousarray(action.T), W_enc,
                            np.eye(128, dtype=f32)], axis=1)
    blobW = np.concatenate([padu(WuA), padu(Wuh), WvA, Wvh], axis=1)
    state_f = state.astype(f32)
    fside = np.zeros((128, FS_COLS), f32)
    fside[:, 0] = -state_f[0:128, 0]
    fside[:, 1] = -state_f[0:128, 1]
    fside[:, 2] = -state_f[128:256, 0]
    fside[:, 3] = -state_f[128:256, 1]
    fside[:, 4] = np.arange(128, dtype=f32)
    fside[:, 5] = np.arange(128, 256, dtype=f32)

    wbig8 = np.zeros((128, 48), f32)
    for h in range(4):
        wbig8[:, 12 * h:12 * h + 6] = Wbig[144 * h:144 * h + 128]
        wbig8[0:16, 12 * h + 6:12 * h + 12] = Wbig[144 * h + 128:144 * (h + 1)]
    nbrow = np.zeros((128, 12), f32)
    nbrow[0, 0:6] = nvec
    nbrow[0, 6:12] = bva6

    shared = {
        "blobA": blobA.astype(BF),
        "blobW": blobW.astype(BF),
        "fside": fside,
    }
    in_maps = []
    for c in range(NCORES):
        bF = np.concatenate([wbig8, nbrow], axis=1)
        crow = np.stack([state_f[c * SH:(c + 1) * SH, 0],
                         state_f[c * SH:(c + 1) * SH, 1],
                         np.arange(c * SH, (c + 1) * SH, dtype=f32)])
        m = dict(shared)
        m["blobF"] = np.ascontiguousarray(bF).astype(BF)
        m["crow"] = np.ascontiguousarray(crow).astype(BF)
        in_maps.append(m)
    return in_maps


def _zero_qk_bias(inputs):
    return all(not np.any(np.asarray(inputs[k]))
               for k in ("bq", "bk", "biq", "bik"))


def kernel(**inputs):
    if not _zero_qk_bias(inputs):
        return _kernel_general(inputs)
    if "fast" not in _NC_CACHE:
        _NC_CACHE["fast"] = _build_fast()
    nc = _NC_CACHE["fast"]
    in_maps = _make_in_maps_fast(inputs)
    res = bass_utils.run_bass_kernel_spmd(nc, in_maps,
                                          core_ids=list(range(NCORES)))
    return np.concatenate([res.results[c]["out"] for c in range(NCORES)],
                          axis=0)


# ======================= general path (nonzero q/k biases) ==================
EPg = 640
QKMg = [(0, 128), (128, 128), (256, 128), (384, 128), (512, 64), (576, 64)]
GW_QA, GW_QH, GW_KA, GW_KH, GW_VA, GW_VH = (0, EPg, 2 * EPg, 3 * EPg,
                                            4 * EPg, 4 * EPg + E)
GW_COLS = 4 * EPg + 2 * E
GF_SEL, GF_WB = 0, 64
GF_COLS = GF_WB + 48
GFS_NEG, GFS_BQ, GFS_BK = 0, 4, 10
GFS_COLS = 16


def _build_general():
    nc = bacc.Bacc(target_bir_lowering=False)

    def dp(name, shape, dtype, isOutput=False):
        return nc.declare_dram_parameter(name, shape, dtype, isOutput)

    blobA_d = dp("blobA", [128, 512], BF16)
    blobW_d = dp("blobW", [128, GW_COLS], BF16)
    blobF_d = dp("blobF", [128, GF_COLS], BF16)
    fside_d = dp("fside", [128, GFS_COLS], F32)
    srows_d = dp("srows", [2, N], BF16)
    nb2_d = dp("nb2", [1, 12], F32)
    out_d = dp("out", [SH, ACT], F32, isOutput=True)

    with TileContext(nc) as tc:
        with contextlib.ExitStack() as ctx:
            wp = ctx.enter_context(tc.tile_pool(name="wp", bufs=1))
            pp = ctx.enter_context(tc.tile_pool(name="pp", bufs=7, space="PSUM"))

            def wt(shape, tag, dtype=BF16):
                return wp.tile(shape, dtype, tag=tag, name=tag)

            def ps(shape):
                return pp.tile(shape, F32, tag="mm", name="mm")

            dma = nc.sync.dma_start
            sdma = nc.scalar.dma_start
            gdma = nc.gpsimd.dma_start

            blobA = wt([128, 512], "blobA")
            dma(out=blobA, in_=blobA_d[:, :])
            blobW = wt([128, GW_COLS], "blobW")
            dma(out=blobW[:, 0:2 * EPg], in_=blobW_d[:, 0:2 * EPg])
            fside = wt([128, GFS_COLS], "fside", F32)
            sdma(out=fside, in_=fside_d[:, :])
            sdma(out=blobW[:, 2 * EPg:4 * EPg], in_=blobW_d[:, 2 * EPg:4 * EPg])
            dma(out=blobW[:, 4 * EPg:GW_COLS], in_=blobW_d[:, 4 * EPg:GW_COLS])
            blobF = wt([128, GF_COLS], "blobF")
            sdma(out=blobF, in_=blobF_d[:, :])
            xjf = wt([128, N], "xjf")
            gdma(out=xjf, in_=bass.AP(tensor=srows_d.ap().tensor, offset=0,
                                      ap=[[0, 128], [1, N]]))
            yjf = wt([128, N], "yjf")
            gdma(out=yjf, in_=bass.AP(tensor=srows_d.ap().tensor, offset=N,
                                      ap=[[0, 128], [1, N]]))
            nb_bc = wt([SH, 12], "nbbc", F32)
            gdma(out=nb_bc, in_=bass.AP(tensor=nb2_d.ap().tensor, offset=0,
                                        ap=[[0, SH], [1, 12]]))

            hT = blobA[:, 0:256]
            aT = blobA[:, 256:512]
            movs = [aT, hT]
            wq_t = [blobW[:, GW_QA:GW_QA + EPg], blobW[:, GW_QH:GW_QH + EPg]]
            wk_t = [blobW[:, GW_KA:GW_KA + EPg], blobW[:, GW_KH:GW_KH + EPg]]
            wv_t = [blobW[:, GW_VA:GW_VA + E], blobW[:, GW_VH:GW_VH + E]]
            sel_t = [blobF[:, GF_SEL:GF_SEL + SH],
                     blobF[:, GF_SEL + SH:GF_SEL + 2 * SH]]
            wbig_t = [blobF[:, GF_WB + 6 * g:GF_WB + 6 * g + 6]
                      for g in range(8)]
            negx = [fside[:, 0:1], fside[:, 2:3]]
            negy = [fside[:, 1:2], fside[:, 3:4]]
            bq_t = [fside[:, GFS_BQ + h:GFS_BQ + h + 1] for h in range(4)] + \
                   [fside[0:64, GFS_BQ + 4:GFS_BQ + 5],
                    fside[0:64, GFS_BQ + 5:GFS_BQ + 6]]
            bk_t = [fside[:, GFS_BK + h:GFS_BK + h + 1] for h in range(4)] + \
                   [fside[0:64, GFS_BK + 4:GFS_BK + 5],
                    fside[0:64, GFS_BK + 5:GFS_BK + 6]]

            mask_t = []
            for c in range(2):
                bx = wt([128, N], f"bx{c}")
                by = wt([128, N], f"by{c}")
                nc.vector.tensor_scalar(bx, xjf, negx[c], None,
                                        mybir.AluOpType.add)
                nc.vector.tensor_scalar(by, yjf, negy[c], None,
                                        mybir.AluOpType.add)
                bx2 = wt([128, N], f"bx2{c}")
                by2 = wt([128, N], f"by2{c}")
                nc.vector.tensor_tensor(bx2, bx, bx, mybir.AluOpType.mult)
                nc.vector.tensor_tensor(by2, by, by, mybir.AluOpType.mult)
                nc.vector.tensor_scalar(bx, bx2, 16.0, None,
                                        mybir.AluOpType.is_le)
                nc.vector.tensor_scalar(by, by2, 4.0, None,
                                        mybir.AluOpType.is_le)
                prox = wt([128, N], f"prox{c}")
                nc.vector.tensor_tensor(prox, bx, by, mybir.AluOpType.mult)
                mk = wt([128, N], f"mask{c}")
                nc.gpsimd.affine_select(out=mk, in_=prox, pattern=[[1, N]],
                                        compare_op=mybir.AluOpType.is_gt,
                                        fill=0.0, base=-c * 128,
                                        channel_multiplier=-1)
                mask_t.append(mk)

            def proj(w_t, b_t, tag):
                outs = []
                for mi, (ms, ml) in enumerate(QKMg):
                    p = ps([ml, N])
                    for ci in range(2):
                        nc.tensor.matmul(p, w_t[ci][:, ms:ms + ml], movs[ci],
                                         start=(ci == 0), stop=(ci == 1))
                    t = wt([ml, N], f"{tag}{mi}")
                    nc.vector.tensor_scalar(t, p, b_t[mi], None,
                                            mybir.AluOpType.add)
                    outs.append(t)
                return outs

            qT = proj(wq_t, bq_t, "qT")
            kT = proj(wk_t, bk_t, "kT")

            E_t = [[None, None] for _ in range(NH)]
            ET_t = [[None, None] for _ in range(NH)]
            for h in range(NH):
                hs = [(h, 0, 128), (4 + h // 2, 32 * (h % 2), 32)]
                for mj in range(2):
                    pS = ps([128, N])
                    pST = ps([128, N])
                    for ci, (ti, rs, rl) in enumerate(hs):
                        st_, sp = (ci == 0), (ci == 1)
                        nc.tensor.matmul(
                            pS, qT[ti][rs:rs + rl, mj * 128:(mj + 1) * 128],
                            kT[ti][rs:rs + rl, :], start=st_, stop=sp)
                        nc.tensor.matmul(
                            pST, kT[ti][rs:rs + rl, mj * 128:(mj + 1) * 128],
                            qT[ti][rs:rs + rl, :], start=st_, stop=sp)
                    Eh = wt([128, N], f"E{h}_{mj}")
                    ETh = wt([128, N], f"ET{h}_{mj}")
                    nc.scalar.activation(Eh, pS,
                                         mybir.ActivationFunctionType.Exp,
                                         scale=SCALE)
                    nc.scalar.activation(ETh, pST,
                                         mybir.ActivationFunctionType.Exp,
                                         scale=SCALE)
                    E_t[h][mj] = Eh
                    ET_t[h][mj] = ETh

            v_t = []
            for nt in range(2):
                vt = wt([128, E], f"v{nt}")
                for ns, nl in ((0, 288), (288, 288)):
                    p = ps([128, nl])
                    for ci in range(2):
                        nc.tensor.matmul(
                            p, movs[ci][:, nt * 128:(nt + 1) * 128],
                            wv_t[ci][:, ns:ns + nl],
                            start=(ci == 0), stop=(ci == 1))
                    nc.any.tensor_copy(out=vt[:, ns:ns + nl], in_=p)
                v_t.append(vt)

            mcT = []
            for km in range(2):
                p = ps([128, SH])
                for c in range(2):
                    nc.tensor.matmul(
                        p, mask_t[c][:, km * 128:(km + 1) * 128],
                        sel_t[c], start=(c == 0), stop=(c == 1))
                t = wt([128, SH], f"mcT{km}")
                nc.any.tensor_copy(out=t, in_=p)
                mcT.append(t)
            ones_t = wt([128, 1], "ones_t")
            nc.vector.memset(ones_t, 1.0)
            pn = ps([SH, 1])
            for c in range(2):
                nc.tensor.matmul(pn, mcT[c], ones_t,
                                 start=(c == 0), stop=(c == 1))
            n_i = wt([SH, 1], "n_i", F32)
            nc.any.tensor_copy(out=n_i, in_=pn)

            RT = {}
            for h in range(NH):
                for jm in range(2):
                    p = ps([128, SH])
                    for kc in range(2):
                        nc.tensor.matmul(
                            p, ET_t[h][kc][:, jm * 128:(jm + 1) * 128],
                            mcT[kc], start=(kc == 0), stop=(kc == 1))
                    rtf = wt([128, SH], f"RTf{h}_{jm}", F32)
                    nc.vector.tensor_scalar(rtf, p, 1e-9, None,
                                            mybir.AluOpType.max)
                    with nc.allow_low_precision(reason="attn renorm"):
                        nc.vector.reciprocal(rtf, rtf)
                    rt = wt([128, SH], f"RT{h}_{jm}")
                    nc.vector.tensor_tensor(rt, rtf, mcT[jm],
                                            mybir.AluOpType.mult)
                    RT[(h, jm)] = rt
            WT = {}
            for h in range(NH):
                for km in range(2):
                    p = ps([128, SH])
                    for jc in range(2):
                        nc.tensor.matmul(
                            p, E_t[h][jc][:, km * 128:(km + 1) * 128],
                            RT[(h, jc)], start=(jc == 0), stop=(jc == 1))
                    wtl = wt([128, SH], f"WT{h}_{km}")
                    nc.vector.tensor_tensor(wtl, p, mcT[km],
                                            mybir.AluOpType.mult)
                    WT[(h, km)] = wtl
            ctxT8 = [None] * 8
            for h in range(NH):
                for dm, (ds, dl) in enumerate([(0, 128), (128, 16)]):
                    p = ps([dl, SH])
                    for kc in range(2):
                        nc.tensor.matmul(
                            p, v_t[kc][:, HD * h + ds:HD * h + ds + dl],
                            WT[(h, kc)], start=(kc == 0), stop=(kc == 1))
                    t = wt([dl, SH], f"cT{2 * h + dm}")
                    nc.any.tensor_copy(out=t, in_=p)
                    ctxT8[2 * h + dm] = t

            pVA = ps([SH, 6])
            for g in range(8):
                dl = 128 if g % 2 == 0 else 16
                nc.tensor.matmul(pVA, ctxT8[g], wbig_t[g][0:dl, :],
                                 start=(g == 0), stop=(g == 7))
            VAt = wt([SH, 6], "VAt", F32)
            nc.vector.scalar_tensor_tensor(
                out=VAt, in0=nb_bc[:, 0:6], scalar=n_i, in1=pVA,
                op0=mybir.AluOpType.mult, op1=mybir.AluOpType.add)
            VA = wt([SH, 6], "VA", F32)
            nc.vector.tensor_tensor(VA, VAt, nb_bc[:, 6:12],
                                    mybir.AluOpType.add)
            sA = wt([SH, 1], "sA", F32)
            nc.vector.reduce_sum(sA, VA[:, 1:6], axis=mybir.AxisListType.X)
            vm = wt([SH, 1], "vm", F32)
            nc.vector.scalar_tensor_tensor(out=vm, in0=sA, scalar=-0.2,
                                           in1=VA[:, 0:1],
                                           op0=mybir.AluOpType.mult,
                                           op1=mybir.AluOpType.add)
            Q_sb = wt([SH, ACT], "Qsb", F32)
            nc.vector.tensor_scalar(Q_sb, VA[:, 1:6], vm, None,
                                    mybir.AluOpType.add)
            nc.gpsimd.dma_start(out=out_d[:, :], in_=Q_sb)

    nc.compile()
    return nc


def _make_in_maps_general(inputs):
    f32 = np.float32
    g = lambda k: np.asarray(inputs[k], dtype=f32)

    hidden, action = g("hidden_state_n"), g("action_n")
    state = np.asarray(inputs["state_n"]).astype(np.int32)
    W_enc, b_enc = g("W_enc"), g("b_enc")
    Wiq, Wik, Wiv = g("Wiq"), g("Wik"), g("Wiv")

    def fuse(Wo_, bo_, Wi_, bi_):
        Wf = Wo_ @ Wi_
        WA = Wf[16:144]
        Wh = W_enc @ Wf[0:16]
        bf = b_enc @ Wf[0:16] + bo_ @ Wi_ + bi_
        return WA, Wh, bf

    WqA, Wqh, bqf = fuse(g("Wq"), g("bq"), Wiq, g("biq"))
    WkA, Wkh, bkf = fuse(g("Wk"), g("bk"), Wik, g("bik"))
    WvA, Wvh, bvf = fuse(g("Wv"), g("bv"), Wiv, g("biv"))

    Wva6 = np.concatenate([g("W_val").reshape(D, 1),
                           g("W_adv").reshape(D, ACT)], axis=1)
    WoWO = g("Wo_proj") @ g("W_O")
    Wbig = WoWO @ Wva6
    nvec = bvf @ Wbig + (g("bo_proj") @ g("W_O")) @ Wva6
    bva6 = np.concatenate([g("b_val").reshape(1), g("b_adv")])

    def padqk(w):
        mains = [w[:, 144 * h:144 * h + 128] for h in range(4)]
        z = np.zeros((w.shape[0], 16), f32)
        tails = [np.concatenate([w[:, 144 * h + 128:144 * h + 144], z,
                                 w[:, 144 * (h + 1) + 128:144 * (h + 1) + 144],
                                 z], axis=1) for h in (0, 2)]
        return np.concatenate(mains + tails, axis=1)

    def bias_cols(b):
        cols = np.zeros((128, 6), f32)
        for h in range(4):
            cols[:, h] = b[144 * h:144 * h + 128]
        for t, h in enumerate((0, 2)):
            cols[0:16, 4 + t] = b[144 * h + 128:144 * h + 144]
            cols[32:48, 4 + t] = b[144 * (h + 1) + 128:144 * (h + 1) + 144]
        return cols

    blobA = np.concatenate([np.ascontiguousarray(hidden.T),
                            np.ascontiguousarray(action.T)], axis=1)
    blobW = np.concatenate([padqk(WqA), padqk(Wqh), padqk(WkA), padqk(Wkh),
                            WvA, Wvh], axis=1)
    state_f = state.astype(f32)
    fside = np.zeros((128, GFS_COLS), f32)
    fside[:, 0] = -state_f[0:128, 0]
    fside[:, 1] = -state_f[0:128, 1]
    fside[:, 2] = -state_f[128:256, 0]
    fside[:, 3] = -state_f[128:256, 1]
    fside[:, GFS_BQ:GFS_BQ + 6] = bias_cols(bqf)
    fside[:, GFS_BK:GFS_BK + 6] = bias_cols(bkf)
    srows = np.ascontiguousarray(state_f.T)
    nb2 = np.concatenate([nvec, bva6]).reshape(1, 12).astype(f32)

    wbig8 = np.zeros((128, 48), f32)
    for h in range(4):
        wbig8[:, 12 * h:12 * h + 6] = Wbig[144 * h:144 * h + 128]
        wbig8[0:16, 12 * h + 6:12 * h + 12] = Wbig[144 * h + 128:144 * (h + 1)]

    eye = np.eye(N, dtype=f32)
    shared = {
        "blobA": blobA.astype(BF),
        "blobW": blobW.astype(BF),
        "fside": fside,
        "srows": srows.astype(BF),
        "nb2": nb2,
    }
    in_maps = []
    for c in range(NCORES):
        sel = eye[:, c * SH:(c + 1) * SH]
        selpack = np.concatenate([sel[0:128], sel[128:256]], axis=1)
        bF = np.concatenate([selpack, wbig8], axis=1)
        m = dict(shared)
        m["blobF"] = np.ascontiguousarray(bF).astype(BF)
        in_maps.append(m)
    return in_maps


def _kernel_general(inputs):
    if "gen" not in _NC_CACHE:
        _NC_CACHE["gen"] = _build_general()
    nc = _NC_CACHE["gen"]
    in_maps = _make_in_maps_general(inputs)
    res = bass_utils.run_bass_kernel_spmd(nc, in_maps,
                                          core_ids=list(range(NCORES)))
    return np.concatenate([res.results[c]["out"] for c in range(NCORES)],
                          axis=0)
